# revision 1
# baseline (speedup 1.0000x reference)
"""Trainium2 Bass kernel for CompositionalAttentionBase.

Problem (per batch element b, reference semantics):
  q = (x @ Wq + bq)  -> [T,H,P] * 1/sqrt(P)
  k = (x @ Wk + bk)  -> [T,H,P]
  v = (x @ Wv + bv)  -> [T,H,R,P]
  score = softmax(q k^T) per head            [H,Tq,Tk]
  out   = score @ v per (head, rule)         [T,H,R,P]
  q_v = (x @ Wqv + bqv)/sqrt(QK)             [T,H,QK]
  k_v = out @ Wkv + bkv                      [T,H,R,QK]
  comp = softmax_r(q_v . k_v)                [T,H,R]
  out2 = sum_r comp * out                    [T,H,P]
  y = out2.reshape(T,D) @ Wm

Sharding: pure data-parallel over batch. B == n_cores == 8, so each
NeuronCore computes one full batch element; no collectives at all.

Per-core dataflow (head-by-head, everything in "transposed" layouts so
every contraction is a natural TensorE matmul; fp32r everywhere on the
PE for 4x throughput over fp32):
  xT   [D,T]   via PE-transpose of x
  qT_h = Wq_h^T @ xT        [P,T]     (Wq pre-scaled by 1/sqrt(P) on host)
  kT_h = Wk_h^T @ xT        [P,T]
  ET   = exp(kT^T q-slices) [Tk,Tq]   (scores are O(3): softmax without
                                       max-subtraction is exact enough; kept
                                       unnormalized)
  V_h  = x @ Wv_h           [Tk,R*P]
  OTu_r = V_r^T @ ET        [P,Tq]    (unnormalized attention out, per rule)
  ZRep8 = ones8^T @ ET      [8,Tq]    (softmax denominator, replicated)
  qvT  = Wqv_h^T @ xT (4x row-replicated) [4*QK,T]
  kvT  = Wkv^T @ OTu_r (col-tiled, 4 rules/psum) [4*QK,Tq]
  compU = blockdiag-sums of (kvT * qvT)   [8,Tq]
  comp logits = compU / Z;  compE = exp(.)
  w = compE / (CZ * Z)  where CZ = sum_r compE
  out2T_h = sum_r OTu_r * broadcast(w_r)  [P,T]
  y = sum_h out2T_h^T @ Wm_h              [T,D]
"""

import numpy as np
import ml_dtypes

import concourse.bass as bass
import concourse.tile as tile
from concourse import bacc, mybir
from concourse.bass_utils import run_bass_kernel_spmd

B, T, D, H, R, QK = 8, 1024, 1024, 8, 8, 32
P = D // H  # 128
NCORES = 8
TT = T // 128  # 8 t-tiles
KT = D // 128  # 8 contraction tiles for D
NC2 = T // 512  # 2 free-dim chunks of 512 over T
F32 = mybir.dt.float32
F32R = mybir.dt.float32r
BF16 = mybir.dt.bfloat16
EXP = mybir.ActivationFunctionType.Exp


def _c(c):  # 512-chunk slice
    return slice(c * 512, (c + 1) * 512)


def _t(i):  # 128-tile slice
    return slice(i * 128, (i + 1) * 128)


def build_kernel(tc, io, flags):
    nc = tc.nc
    x_d = io["x"]
    y_d = io["y"]

    with (
        nc.allow_low_precision(reason="f32r intermediates; end-to-end precision validated vs reference"),
        tc.tile_pool(name="big", bufs=1) as big,
        tc.tile_pool(name="sb", bufs=1) as sb,
        tc.tile_pool(name="wq", bufs=1) as wqp,
        tc.tile_pool(name="stream", bufs=2) as stream,
        tc.tile_pool(name="psA", bufs=3, space="PSUM") as psA,
        tc.tile_pool(name="psV", bufs=4, space="PSUM") as psV,
        tc.tile_pool(name="psS", bufs=1, space="PSUM") as psS,
    ):
        # ---- constants ----
        ident = sb.tile([128, 128], F32R, name="ident")
        nc.sync.dma_start(ident[:], io["c_ident"])
        ones8 = sb.tile([128, 8], F32R, name="ones8")
        nc.sync.dma_start(ones8[:], io["c_ones8"])
        blkA = sb.tile([128, 8], F32R, name="blkA")
        nc.sync.dma_start(blkA[:], io["c_blkA"])
        blkB = sb.tile([128, 8], F32R, name="blkB")
        nc.sync.dma_start(blkB[:], io["c_blkB"])
        ones88 = sb.tile([8, 8], F32R, name="ones88")
        nc.sync.dma_start(ones88[:], io["c_ones88"])
        sel8 = sb.tile([8, 1024], F32R, name="sel8")
        nc.sync.dma_start(sel8[:], io["c_sel8"])
        wkvblk = sb.tile([128, 4, 128], F32R, name="wkvblk")
        nc.sync.dma_start(wkvblk[:], io["c_wkvblk"].rearrange("p (g m) -> p g m", g=4))
        if flags["bq"]:
            bq_sb = sb.tile([128, 8], F32R, name="bq_sb")
            nc.sync.dma_start(bq_sb[:], io["bq"].rearrange("(h p) -> p h", p=128))
        if flags["bk"]:
            bk_sb = sb.tile([128, 8], F32R, name="bk_sb")
            nc.sync.dma_start(bk_sb[:], io["bk"].rearrange("(h p) -> p h", p=128))
        if flags["bqv"]:
            # per head h: [32] -> replicated 4x on partitions
            bqv_sb = sb.tile([128, 8], F32R, name="bqv_sb")
            nc.sync.dma_start(
                bqv_sb[:],
                io["bqv"].rearrange("(h q) -> q h", q=32).to_broadcast([4, 32, 8]).rearrange("r q h -> (r q) h"),
            )
        if flags["bv"]:
            onesrow = sb.tile([1, 128], F32R, name="onesrow")
            nc.sync.dma_start(onesrow[:], io["c_onesrow"])
        if flags["bkv"]:
            bkv_sb = sb.tile([128, 1], F32R, name="bkv_sb")
            nc.sync.dma_start(
                bkv_sb[:], io["bkv"].rearrange("(o q) -> q o", o=1).to_broadcast([4, 32, 1]).rearrange("r q o -> (r q) o")
            )

        # ---- x load + transpose -> xT [128(d), KT, T] ----
        xload = big.tile([128, TT, D], F32R, tag="ET", name="xload")
        for tt in range(TT):
            nc.sync.dma_start(xload[:, tt], x_d[_t(tt), :])
        xT = big.tile([128, KT, T], F32R, tag="xT", name="xT")
        for c in range(KT):
            for tt in range(TT):
                pst = psA.tile([128, 128], F32R, tag="acc", name=f"pst{c}_{tt}")
                nc.tensor.transpose(pst[:], xload[:, tt, _t(c)], ident[:])
                nc.scalar.copy(xT[:, c, _t(tt)], pst[:])

        out2 = big.tile([128, H, T], BF16, tag="out2", name="out2")

        for h in range(H):
            # ---- load per-head weights ----
            wqh = wqp.tile([128, KT, 128], F32R, tag="wq", name=f"wq{h}")
            wkh = wqp.tile([128, KT, 128], F32R, tag="wk", name=f"wk{h}")
            for kt in range(KT):
                nc.sync.dma_start(wqh[:, kt], io["Wq"][_t(kt), _t(h)])
                nc.sync.dma_start(wkh[:, kt], io["Wk"][_t(kt), _t(h)])
            wqvh = wqp.tile([128, KT, 128], F32R, tag="wqv", name=f"wqv{h}")
            for kt in range(KT):
                for rep in range(4):
                    nc.sync.dma_start(
                        wqvh[:, kt, rep * 32 : (rep + 1) * 32],
                        io["Wqv"][_t(kt), h * 32 : (h + 1) * 32],
                    )

            # ---- qT / kT ----
            qT = big.tile([128, T], F32R, tag="qT", name=f"qT{h}")
            kT = big.tile([128, T], F32R, tag="kT", name=f"kT{h}")
            for dst, w, bflag, bias in (
                (qT, wqh, flags["bq"], "bq_sb"),
                (kT, wkh, flags["bk"], "bk_sb"),
            ):
                for c in range(NC2):
                    ps = psA.tile([128, 512], F32, tag="acc", name=f"psqk{h}_{c}")
                    for kt in range(KT):
                        nc.tensor.matmul(
                            ps[:], w[:, kt], xT[:, kt, _c(c)],
                            start=(kt == 0), stop=(kt == KT - 1),
                        )
                    if bflag:
                        nc.scalar.activation(
                            dst[:, _c(c)], ps[:],
                            mybir.ActivationFunctionType.Identity,
                            bias=(bq_sb if dst is qT else bk_sb)[:, h : h + 1],
                        )
                    else:
                        nc.scalar.copy(dst[:, _c(c)], ps[:])

            # ---- ET = exp(scores^T) [128(tk), TT, T(q)] ----
            ET = big.tile([128, TT, T], F32R, tag="ET", name=f"ET{h}")
            for tk in range(TT):
                for c in range(NC2):
                    ps = psA.tile([128, 512], F32, tag="acc", name=f"pse{h}_{tk}_{c}")
                    nc.tensor.matmul(ps[:], kT[:, _t(tk)], qT[:, _c(c)], start=True, stop=True)
                    nc.scalar.activation(ET[:, tk, _c(c)], ps[:], EXP)

            # ---- V [128(tk), TT, R*P chunk] ----
            V = big.tile([128, TT, 1024], F32R, tag="V", name=f"V{h}")
            for tg in range(2):
                for c in range(2):  # chunk of R*P (1024)
                    pv = [
                        psV.tile([128, 512], F32, tag="vacc", name=f"psv{h}_{tg}_{c}_{i}")
                        for i in range(4)
                    ]
                    for kt in range(KT):
                        wv_t = stream.tile([128, 512], F32R, tag="wv", name=f"wv{h}_{tg}_{c}_{kt}")
                        nc.sync.dma_start(wv_t[:], io["Wv"][_t(kt), h * 1024 + c * 512 : h * 1024 + (c + 1) * 512])
                        for i in range(4):
                            tt = tg * 4 + i
                            nc.tensor.matmul(
                                pv[i][:], xT[:, kt, _t(tt)], wv_t[:],
                                start=(kt == 0), stop=(kt == KT - 1 and not flags["bv"]),
                            )
                    if flags["bv"]:
                        bv_t = stream.tile([1, 512], F32R, tag="bv", name=f"bv{h}_{tg}_{c}")
                        nc.sync.dma_start(bv_t[:], io["bv"][None, h * 1024 + c * 512 : h * 1024 + (c + 1) * 512])
                        for i in range(4):
                            nc.tensor.matmul(pv[i][:], onesrow[:], bv_t[:], start=False, stop=True)
                    for i in range(4):
                        tt = tg * 4 + i
                        nc.vector.tensor_copy(V[:, tt, _c(c)], pv[i][:])

            # ---- OTu_r = V_r^T @ ET  [128(p), R, T(q)] ----
            OTu = big.tile([128, R, T], F32R, tag="OTu", name=f"OTu{h}")
            for r in range(R):
                for c in range(NC2):
                    po = psA.tile([128, 512], F32, tag="acc", name=f"pso{h}_{r}_{c}")
                    for tk in range(TT):
                        nc.tensor.matmul(
                            po[:], V[:, tk, _t(r)], ET[:, tk, _c(c)],
                            start=(tk == 0), stop=(tk == TT - 1),
                        )
                    nc.scalar.copy(OTu[:, r, _c(c)], po[:])

            # ---- ZRep8 + recipZ ----
            recipZ = sb.tile([8, T], F32R, tag="recipZ", name=f"recipZ{h}")
            for c in range(NC2):
                pz = psS.tile([8, 512], F32, tag="small", name=f"psz{h}_{c}")
                for tk in range(TT):
                    nc.tensor.matmul(
                        pz[:], ones8[:], ET[:, tk, _c(c)],
                        start=(tk == 0), stop=(tk == TT - 1),
                    )
                nc.vector.reciprocal(recipZ[:, _c(c)], pz[:])

            # ---- qvRep [128(4x qk), T] ----
            qvRep = sb.tile([128, T], F32R, tag="qvRep", name=f"qvRep{h}")
            for c in range(NC2):
                pq = psA.tile([128, 512], F32, tag="acc", name=f"psq{h}_{c}")
                for kt in range(KT):
                    nc.tensor.matmul(
                        pq[:], wqvh[:, kt], xT[:, kt, _c(c)],
                        start=(kt == 0), stop=(kt == KT - 1),
                    )
                if flags["bqv"]:
                    nc.scalar.activation(
                        qvRep[:, _c(c)], pq[:],
                        mybir.ActivationFunctionType.Identity,
                        bias=bqv_sb[:, h : h + 1],
                    )
                else:
                    nc.scalar.copy(qvRep[:, _c(c)], pq[:])

            # ---- kvT (col-tiled 4 rules / psum tile) + P-mul ----
            PP = sb.tile([128, 2, T], F32R, tag="PP", name=f"PP{h}")
            for c in range(NC2):
                for g in range(2):
                    pk = psA.tile([128, 512], F32, tag="acc", name=f"psk{h}_{c}_{g}")
                    for rr in range(4):
                        r = g * 4 + rr
                        nc.tensor.matmul(
                            pk[:], wkvblk[:, rr], OTu[:, r, _c(c)],
                            start=(rr == 0), stop=(rr == 3),
                        )
                    if flags["bkv"]:
                        tmp = sb.tile([128, 512], F32R, tag="kvtmp", name=f"kvt{h}_{c}_{g}", bufs=2)
                        nc.vector.tensor_scalar_add(tmp[:], pk[:], bkv_sb[:, 0:1])
                        nc.vector.tensor_tensor(PP[:, g, _c(c)], tmp[:], qvRep[:, _c(c)], op=mybir.AluOpType.mult)
                    else:
                        nc.vector.tensor_tensor(PP[:, g, _c(c)], pk[:], qvRep[:, _c(c)], op=mybir.AluOpType.mult)

            # ---- compU -> compL -> compE ----
            compE = sb.tile([8, T], F32R, tag="compE", name=f"compE{h}")
            for c in range(NC2):
                pc = psS.tile([8, 512], F32, tag="small", name=f"psc{h}_{c}")
                nc.tensor.matmul(pc[:], blkA[:], PP[:, 0, _c(c)], start=True, stop=False)
                nc.tensor.matmul(pc[:], blkB[:], PP[:, 1, _c(c)], start=False, stop=True)
                compL = sb.tile([8, 512], F32R, tag="compL", name=f"compL{h}_{c}", bufs=1)
                nc.vector.tensor_tensor(compL[:], pc[:], recipZ[:, _c(c)], op=mybir.AluOpType.mult)
                nc.scalar.activation(compE[:, _c(c)], compL[:], EXP)

            # ---- CZ -> w8 ----
            w8 = sb.tile([8, T], F32R, tag="w8", name=f"w8{h}")
            for c in range(NC2):
                pcz = psS.tile([8, 512], F32, tag="small", name=f"pscz{h}_{c}")
                nc.tensor.matmul(pcz[:], ones88[:], compE[:, _c(c)], start=True, stop=True)
                recipCZ = sb.tile([8, 512], F32R, tag="recipCZ", name=f"rcz{h}_{c}", bufs=1)
                nc.vector.reciprocal(recipCZ[:], pcz[:])
                denom = sb.tile([8, 512], F32R, tag="denom", name=f"den{h}_{c}", bufs=1)
                nc.vector.tensor_tensor(denom[:], recipCZ[:], recipZ[:, _c(c)], op=mybir.AluOpType.mult)
                nc.vector.tensor_tensor(w8[:, _c(c)], compE[:, _c(c)], denom[:], op=mybir.AluOpType.mult)

            # ---- wRep broadcast (PE select-matmul) + weighted sum over rules ----
            for r in range(R):
                for c in range(NC2):
                    wr_ps = psA.tile([128, 512], F32, tag="acc", name=f"wrps{h}_{r}_{c}")
                    nc.tensor.matmul(wr_ps[:], sel8[:, _t(r)], w8[:, _c(c)], start=True, stop=True)
                    nc.vector.tensor_tensor(
                        OTu[:, r, _c(c)], OTu[:, r, _c(c)], wr_ps[:], op=mybir.AluOpType.mult
                    )
            nc.vector.tensor_reduce(
                out2[:, h, :],
                OTu[:].rearrange("p r t -> p t r"),
                axis=mybir.AxisListType.X,
                op=mybir.AluOpType.add,
            )

        # ---- merge: y = sum_h out2_h^T @ Wm_h ----
        wm = big.tile([128, H, D], BF16, tag="V", name="wm")
        for h in range(H):
            for c in range(NC2):
                nc.sync.dma_start(wm[:, h, _c(c)], io["Wm"][_t(h), _c(c)])
        for tt in range(TT):
            for c in range(NC2):
                py = psA.tile([128, 512], F32, tag="acc", name=f"psy{tt}_{c}")
                for h in range(H):
                    nc.tensor.matmul(
                        py[:], out2[:, h, _t(tt)], wm[:, h, _c(c)],
                        start=(h == 0), stop=(h == H - 1),
                    )
                yt = sb.tile([128, 512], F32, tag="yt", name=f"yt{tt}_{c}", bufs=1)
                nc.scalar.copy(yt[:], py[:])
                nc.sync.dma_start(y_d[_t(tt), _c(c)], yt[:])


_CACHE = {}


def _build(flags_key):
    if flags_key in _CACHE:
        return _CACHE[flags_key]
    flags = dict(flags_key)
    nc = bacc.Bacc("TRN2", target_bir_lowering=False, debug=False, num_devices=NCORES)
    io = {}
    io["x"] = nc.dram_tensor("x", [T, D], F32R, kind="ExternalInput").ap()
    io["Wq"] = nc.dram_tensor("Wq", [D, D], F32R, kind="ExternalInput").ap()
    io["Wk"] = nc.dram_tensor("Wk", [D, D], F32R, kind="ExternalInput").ap()
    io["Wv"] = nc.dram_tensor("Wv", [D, H * R * P], F32R, kind="ExternalInput").ap()
    io["Wqv"] = nc.dram_tensor("Wqv", [D, H * QK], F32R, kind="ExternalInput").ap()
    io["c_wkvblk"] = nc.dram_tensor("c_wkvblk", [128, 512], F32R, kind="ExternalInput").ap()
    io["Wm"] = nc.dram_tensor("Wm", [D, D], BF16, kind="ExternalInput").ap()
    for bname, shape in (
        ("bq", [D]), ("bk", [D]), ("bv", [H * R * P]), ("bqv", [H * QK]), ("bkv", [QK]),
    ):
        if flags[bname]:
            io[bname] = nc.dram_tensor(bname, shape, F32R, kind="ExternalInput").ap()
    io["c_ident"] = nc.dram_tensor("c_ident", [128, 128], F32R, kind="ExternalInput").ap()
    io["c_ones8"] = nc.dram_tensor("c_ones8", [128, 8], F32R, kind="ExternalInput").ap()
    io["c_blkA"] = nc.dram_tensor("c_blkA", [128, 8], F32R, kind="ExternalInput").ap()
    io["c_blkB"] = nc.dram_tensor("c_blkB", [128, 8], F32R, kind="ExternalInput").ap()
    io["c_ones88"] = nc.dram_tensor("c_ones88", [8, 8], F32R, kind="ExternalInput").ap()
    io["c_sel8"] = nc.dram_tensor("c_sel8", [8, 1024], F32R, kind="ExternalInput").ap()
    if flags["bv"]:
        io["c_onesrow"] = nc.dram_tensor("c_onesrow", [1, 128], F32R, kind="ExternalInput").ap()
    io["y"] = nc.dram_tensor("y", [T, D], F32, kind="ExternalOutput").ap()

    with tile.TileContext(nc) as tc:
        build_kernel(tc, io, flags)
    nc.compile()
    _CACHE[flags_key] = (nc, flags)
    return _CACHE[flags_key]


def _wkvblk(Wkv):
    blk = np.zeros((128, 4, 128), np.float32)
    for g in range(4):
        blk[:, g, g * 32 : (g + 1) * 32] = Wkv
    return np.ascontiguousarray(blk.reshape(128, 512))


def _consts():
    ident = np.eye(128, dtype=np.float32)
    ones8 = np.zeros((128, 8), np.float32)
    for k in range(128):
        ones8[k, :] = 1.0
    blkA = np.zeros((128, 8), np.float32)
    blkB = np.zeros((128, 8), np.float32)
    for k in range(128):
        g = k // 32
        blkA[k, g] = 1.0
        blkB[k, 4 + g] = 1.0
    ones88 = np.ones((8, 8), np.float32)
    onesrow = np.ones((1, 128), np.float32)
    sel8 = np.zeros((8, 1024), np.float32)
    for m in range(1024):
        sel8[m // 128, m] = 1.0
    return {
        "c_ident": ident, "c_ones8": ones8, "c_blkA": blkA,
        "c_blkB": blkB, "c_ones88": ones88, "c_onesrow": onesrow,
        "c_sel8": sel8,
    }


def kernel(**inputs):
    inp = {k: np.ascontiguousarray(np.asarray(v, dtype=np.float32)) for k, v in inputs.items()}
    flags = {b: bool(np.any(inp[b])) for b in ("bq", "bk", "bv", "bqv", "bkv")}
    flags_key = tuple(sorted(flags.items()))
    nc, flags = _build(flags_key)

    scale_q = np.float32(1.0 / np.sqrt(P))
    scale_qv = np.float32(1.0 / np.sqrt(QK))
    Wq = inp["Wq"] * scale_q
    Wqv = inp["Wqv"] * scale_qv
    consts = _consts()
    base = {
        "Wq": Wq, "Wk": inp["Wk"], "Wv": inp["Wv"], "Wqv": Wqv,
        "c_wkvblk": _wkvblk(inp["Wkv"]), "Wm": inp["Wm"].astype(ml_dtypes.bfloat16),
        "c_ident": consts["c_ident"], "c_ones8": consts["c_ones8"],
        "c_blkA": consts["c_blkA"], "c_blkB": consts["c_blkB"],
        "c_ones88": consts["c_ones88"], "c_sel8": consts["c_sel8"],
    }
    if flags["bq"]:
        base["bq"] = inp["bq"] * scale_q
    if flags["bk"]:
        base["bk"] = inp["bk"]
    if flags["bqv"]:
        base["bqv"] = inp["bqv"] * scale_qv
    if flags["bv"]:
        base["bv"] = inp["bv"]
        base["c_onesrow"] = consts["c_onesrow"]
    if flags["bkv"]:
        base["bkv"] = inp["bkv"]

    in_maps = []
    for c in range(NCORES):
        m = dict(base)
        m["x"] = np.ascontiguousarray(inp["x"][c])
        in_maps.append(m)

    res = run_bass_kernel_spmd(nc, in_maps, list(range(NCORES)))
    out = np.stack([res.results[c]["y"] for c in range(NCORES)], axis=0)
    return out


def run_traced(inputs):
    """Like kernel() but with NTFF tracing; returns (out, BassKernelResults)."""
    inp = {k: np.ascontiguousarray(np.asarray(v, dtype=np.float32)) for k, v in inputs.items()}
    flags = {b: bool(np.any(inp[b])) for b in ("bq", "bk", "bv", "bqv", "bkv")}
    flags_key = tuple(sorted(flags.items()))
    nc, flags = _build(flags_key)
    consts = _consts()
    base = {
        "Wq": inp["Wq"] * np.float32(1.0 / np.sqrt(P)),
        "Wk": inp["Wk"], "Wv": inp["Wv"],
        "Wqv": inp["Wqv"] * np.float32(1.0 / np.sqrt(QK)),
        "c_wkvblk": _wkvblk(inp["Wkv"]), "Wm": inp["Wm"].astype(ml_dtypes.bfloat16),
        "c_ident": consts["c_ident"], "c_ones8": consts["c_ones8"],
        "c_blkA": consts["c_blkA"], "c_blkB": consts["c_blkB"],
        "c_ones88": consts["c_ones88"], "c_sel8": consts["c_sel8"],
    }
    in_maps = []
    for c in range(NCORES):
        m = dict(base)
        m["x"] = np.ascontiguousarray(inp["x"][c])
        in_maps.append(m)
    res = run_bass_kernel_spmd(nc, in_maps, list(range(NCORES)), trace=True)
    out = np.stack([res.results[c]["y"] for c in range(NCORES)], axis=0)
    return out, res



# revision 3
# speedup vs baseline: 1.3526x; 1.3526x over previous
"""Trainium2 Bass kernel for CompositionalAttentionBase.

Problem (per batch element b, reference semantics):
  q = (x @ Wq + bq)  -> [T,H,P] * 1/sqrt(P)
  k = (x @ Wk + bk)  -> [T,H,P]
  v = (x @ Wv + bv)  -> [T,H,R,P]
  score = softmax(q k^T) per head            [H,Tq,Tk]
  out   = score @ v per (head, rule)         [T,H,R,P]
  q_v = (x @ Wqv + bqv)/sqrt(QK)             [T,H,QK]
  k_v = out @ Wkv + bkv                      [T,H,R,QK]
  comp = softmax_r(q_v . k_v)                [T,H,R]
  out2 = sum_r comp * out                    [T,H,P]
  y = out2.reshape(T,D) @ Wm

Sharding: pure data-parallel over batch. B == n_cores == 8, so each
NeuronCore computes one full batch element; no collectives at all.

v2 design notes (vs the fp32r baseline):
  - Everything on the PE is bf16 (PSUM accumulation stays fp32). At
    N=512 the matmul streams at 1 col/cycle for both fp32r and bf16,
    but bf16 enables fast-weight-load (64-cycle LDWEIGHTS, hidden
    behind the 512-cycle matmul) and halves all SBUF/DMA/evacuation
    traffic.
  - x is pre-transposed on the host (xT [D,T]) and all weights are
    pre-packed host-side into per-head contiguous layouts, so every
    weight load is one large DMA and the kernel does zero PE
    transposes.
  - The per-head program is split into stage A (projections, scores,
    V, retrieval OTu, composition logits) and stage B (composition
    softmax tail + rule-weighted sum). B(h-1) is emitted after A(h),
    so the PE never waits on the vector-engine softmax chain at a
    head boundary.
  - The rule-weighted sum uses a contiguous multiply/add chain on
    DVE instead of one big strided tensor_reduce (which measured
    ~15us per head in the baseline trace).

Per-core dataflow (head-by-head; all contractions natural TensorE
matmuls, scores kept unnormalized with 1/Z folded into the final
composition weights):
  qT_h = Wq_h^T @ xT        [P,T]     (Wq pre-scaled by 1/sqrt(P))
  kT_h = Wk_h^T @ xT        [P,T]
  ET   = exp(kT^T q-slices) [Tk,Tq]
  V_h  = xT^T @ Wv_h        [Tk,R*P]
  OTu_r = V_r^T @ ET        [P,Tq]    (unnormalized attention out)
  ZRep8 = ones8^T @ ET      [8,Tq]    -> recipZ
  qvT  = Wqv_h^T @ xT (4x row-replicated) [4*QK,T]
  kvT  = Wkv^T @ OTu_r (block-diag, 4 rules/psum) [4*QK,Tq]
  compU = blockdiag-sums of (kvT * qvT)   [8,Tq]
  compE = exp(compU / Z);  w = compE / (CZ * Z)
  out2_h = sum_r OTu_r * broadcast(w_r)   [P,T]
  y = sum_h out2_h^T @ Wm_h               [T,D]
"""

import numpy as np
import ml_dtypes

import concourse.bass as bass
import concourse.tile as tile
from concourse import bacc, mybir
from concourse.bass_utils import run_bass_kernel_spmd

B, T, D, H, R, QK = 8, 1024, 1024, 8, 8, 32
P = D // H  # 128
NCORES = 8
TT = T // 128  # 8 t-tiles
KT = D // 128  # 8 contraction tiles for D
NC2 = T // 512  # 2 free-dim chunks of 512 over T
F32 = mybir.dt.float32
BF16 = mybir.dt.bfloat16
EXP = mybir.ActivationFunctionType.Exp
MUL = mybir.AluOpType.mult
ADD = mybir.AluOpType.add


def _c(c):  # 512-chunk slice
    return slice(c * 512, (c + 1) * 512)


def _t(i):  # 128-tile slice
    return slice(i * 128, (i + 1) * 128)


def build_kernel(tc, io, flags):
    nc = tc.nc

    with (
        nc.allow_low_precision(reason="bf16 intermediates; end-to-end precision validated vs reference"),
        tc.tile_pool(name="cst", bufs=1) as cst,
        tc.tile_pool(name="per", bufs=1) as per,
        tc.tile_pool(name="hd", bufs=2) as hd,     # double-buffered per-head
        tc.tile_pool(name="hs", bufs=1) as hs,     # single-buffered per-head
        tc.tile_pool(name="sc", bufs=2) as scp,    # small vector scratch
        tc.tile_pool(name="psA", bufs=6, space="PSUM") as psA,
        tc.tile_pool(name="psS", bufs=2, space="PSUM") as psS,
    ):
        # ---- constants ----
        ones8 = cst.tile([128, 8], BF16, name="ones8")
        nc.sync.dma_start(ones8[:], io["c_ones8"])
        blkA = cst.tile([128, 8], BF16, name="blkA")
        nc.sync.dma_start(blkA[:], io["c_blkA"])
        blkB = cst.tile([128, 8], BF16, name="blkB")
        nc.sync.dma_start(blkB[:], io["c_blkB"])
        ones88 = cst.tile([8, 8], BF16, name="ones88")
        nc.sync.dma_start(ones88[:], io["c_ones88"])
        sel8 = cst.tile([8, 1024], BF16, name="sel8")
        nc.sync.dma_start(sel8[:], io["c_sel8"])
        wkvblk = cst.tile([128, 4, 128], BF16, name="wkvblk")
        nc.sync.dma_start(wkvblk[:], io["c_wkvblk"].rearrange("p (g m) -> p g m", g=4))
        if flags["bq"]:
            bq_sb = cst.tile([128, 8], F32, name="bq_sb")
            nc.sync.dma_start(bq_sb[:], io["bq"].rearrange("(h p) -> p h", p=128))
        if flags["bk"]:
            bk_sb = cst.tile([128, 8], F32, name="bk_sb")
            nc.sync.dma_start(bk_sb[:], io["bk"].rearrange("(h p) -> p h", p=128))
        if flags["bqv"]:
            bqv_sb = cst.tile([128, 8], F32, name="bqv_sb")
            nc.sync.dma_start(
                bqv_sb[:],
                io["bqv"].rearrange("(h q) -> q h", q=32).to_broadcast([4, 32, 8]).rearrange("r q h -> (r q) h"),
            )
        if flags["bv"]:
            onesrow = cst.tile([1, 128], BF16, name="onesrow")
            nc.sync.dma_start(onesrow[:], io["c_onesrow"])
        if flags["bkv"]:
            bkv_sb = cst.tile([128, 1], F32, name="bkv_sb")
            nc.sync.dma_start(
                bkv_sb[:], io["bkv"].rearrange("(o q) -> q o", o=1).to_broadcast([4, 32, 1]).rearrange("r q o -> (r q) o")
            )

        # ---- persistent tiles ----
        xT = per.tile([128, KT, T], BF16, name="xT")
        for kt in range(KT):
            nc.sync.dma_start(xT[:, kt], io["xT"][_t(kt), :])
        wm = per.tile([128, H, D], BF16, name="wm")
        nc.sync.dma_start(wm[:], io["WmP"].rearrange("k (h d) -> k h d", h=H))
        out2 = per.tile([128, H, T], BF16, name="out2")

        # ---- per-head weight loads (prefetched one head ahead) ----
        wq = [None] * H
        wk = [None] * H
        wqv = [None] * H
        wv = [None] * H

        def load_weights(h):
            wq[h] = hd.tile([128, D], BF16, tag="wq", name=f"wq{h}")
            nc.sync.dma_start(wq[h][:], io["WqP"][h])
            wk[h] = hd.tile([128, D], BF16, tag="wk", name=f"wk{h}")
            nc.sync.dma_start(wk[h][:], io["WkP"][h])
            wqv[h] = hd.tile([128, D], BF16, tag="wqv", name=f"wqv{h}")
            nc.sync.dma_start(wqv[h][:], io["WqvP"][h])
            wv[h] = hd.tile([128, KT, 1024], BF16, tag="wv", name=f"wv{h}")
            for kt in range(KT):
                nc.sync.dma_start(wv[h][:, kt], io["WvP"][h, kt])

        # per-head state handed from stage A to stage B
        OTu_t = [None] * H
        recipZ_t = [None] * H
        compE_t = [None] * H

        def stage_a(h):
            # ---- qT / kT ----
            qT = hd.tile([128, T], BF16, tag="qT", name=f"qT{h}")
            kT = hd.tile([128, T], BF16, tag="kT", name=f"kT{h}")
            for dst, w, bflag, bname in (
                (qT, wq[h], flags["bq"], "bq"),
                (kT, wk[h], flags["bk"], "bk"),
            ):
                for c in range(NC2):
                    ps = psA.tile([128, 512], F32, tag="acc", name=f"psqk{h}_{c}")
                    for kt in range(KT):
                        nc.tensor.matmul(
                            ps[:], w[:, _t(kt)], xT[:, kt, _c(c)],
                            start=(kt == 0), stop=(kt == KT - 1),
                        )
                    if bflag:
                        nc.scalar.activation(
                            dst[:, _c(c)], ps[:],
                            mybir.ActivationFunctionType.Identity,
                            bias=(bq_sb if dst is qT else bk_sb)[:, h : h + 1],
                        )
                    else:
                        nc.scalar.copy(dst[:, _c(c)], ps[:])

            # ---- ET = exp(scores^T) [128(tk), TT, T(q)] ----
            ET = hs.tile([128, TT, T], BF16, tag="ET", name=f"ET{h}")
            for tk in range(TT):
                for c in range(NC2):
                    ps = psA.tile([128, 512], F32, tag="acc", name=f"pse{h}_{tk}_{c}")
                    nc.tensor.matmul(ps[:], kT[:, _t(tk)], qT[:, _c(c)], start=True, stop=True)
                    nc.scalar.activation(ET[:, tk, _c(c)], ps[:], EXP)

            # ---- V [128(tk), TT, R*P] ----
            V = hs.tile([128, TT, 1024], BF16, tag="V", name=f"V{h}")
            for tt in range(TT):
                for c in range(2):
                    pv = psA.tile([128, 512], F32, tag="acc", name=f"psv{h}_{tt}_{c}")
                    for kt in range(KT):
                        nc.tensor.matmul(
                            pv[:], xT[:, kt, _t(tt)], wv[h][:, kt, _c(c)],
                            start=(kt == 0), stop=(kt == KT - 1 and not flags["bv"]),
                        )
                    if flags["bv"]:
                        bv_t = scp.tile([1, 512], BF16, tag="bv", name=f"bv{h}_{tt}_{c}")
                        nc.sync.dma_start(bv_t[:], io["bv"][None, h * 1024 + c * 512 : h * 1024 + (c + 1) * 512])
                        nc.tensor.matmul(pv[:], onesrow[:], bv_t[:], start=False, stop=True)
                    nc.vector.tensor_copy(V[:, tt, _c(c)], pv[:])

            # ---- OTu_r = V_r^T @ ET  [128(p), R, T(q)] ----
            OTu = hd.tile([128, R, T], BF16, tag="OTu", name=f"OTu{h}")
            OTu_t[h] = OTu
            for r in range(R):
                for c in range(NC2):
                    po = psA.tile([128, 512], F32, tag="acc", name=f"pso{h}_{r}_{c}")
                    for tk in range(TT):
                        nc.tensor.matmul(
                            po[:], V[:, tk, _t(r)], ET[:, tk, _c(c)],
                            start=(tk == 0), stop=(tk == TT - 1),
                        )
                    nc.scalar.copy(OTu[:, r, _c(c)], po[:])

            # ---- Z (softmax denominator) -> recipZ ----
            recipZ = hd.tile([8, T], F32, tag="recipZ", name=f"recipZ{h}")
            recipZ_t[h] = recipZ
            for c in range(NC2):
                pz = psS.tile([8, 512], F32, tag="small", name=f"psz{h}_{c}")
                for tk in range(TT):
                    nc.tensor.matmul(
                        pz[:], ones8[:], ET[:, tk, _c(c)],
                        start=(tk == 0), stop=(tk == TT - 1),
                    )
                nc.vector.reciprocal(recipZ[:, _c(c)], pz[:])

            # ---- qvRep [128(4x qk), T] ----
            qvRep = hs.tile([128, T], BF16, tag="qvRep", name=f"qvRep{h}")
            for c in range(NC2):
                pq = psA.tile([128, 512], F32, tag="acc", name=f"psq{h}_{c}")
                for kt in range(KT):
                    nc.tensor.matmul(
                        pq[:], wqv[h][:, _t(kt)], xT[:, kt, _c(c)],
                        start=(kt == 0), stop=(kt == KT - 1),
                    )
                if flags["bqv"]:
                    nc.scalar.activation(
                        qvRep[:, _c(c)], pq[:],
                        mybir.ActivationFunctionType.Identity,
                        bias=bqv_sb[:, h : h + 1],
                    )
                else:
                    nc.scalar.copy(qvRep[:, _c(c)], pq[:])

            # ---- kvT (4 rules / psum via block-diag Wkv) + P-mul ----
            PP = hs.tile([128, 2, T], BF16, tag="PP", name=f"PP{h}")
            for c in range(NC2):
                for g in range(2):
                    pk = psA.tile([128, 512], F32, tag="acc", name=f"psk{h}_{c}_{g}")
                    for rr in range(4):
                        r = g * 4 + rr
                        nc.tensor.matmul(
                            pk[:], wkvblk[:, rr], OTu[:, r, _c(c)],
                            start=(rr == 0), stop=(rr == 3),
                        )
                    if flags["bkv"]:
                        tmp = scp.tile([128, 512], F32, tag="kvtmp", name=f"kvt{h}_{c}_{g}")
                        nc.vector.tensor_scalar_add(tmp[:], pk[:], bkv_sb[:, 0:1])
                        nc.vector.tensor_tensor(PP[:, g, _c(c)], tmp[:], qvRep[:, _c(c)], op=MUL)
                    else:
                        nc.vector.tensor_tensor(PP[:, g, _c(c)], pk[:], qvRep[:, _c(c)], op=MUL)

            # ---- compU -> comp logits -> compE ----
            compE = hd.tile([8, T], BF16, tag="compE", name=f"compE{h}")
            compE_t[h] = compE
            for c in range(NC2):
                pc = psS.tile([8, 512], F32, tag="small", name=f"psc{h}_{c}")
                nc.tensor.matmul(pc[:], blkA[:], PP[:, 0, _c(c)], start=True, stop=False)
                nc.tensor.matmul(pc[:], blkB[:], PP[:, 1, _c(c)], start=False, stop=True)
                compL = scp.tile([8, 512], F32, tag="compL", name=f"compL{h}_{c}", bufs=1)
                nc.vector.tensor_tensor(compL[:], pc[:], recipZ[:, _c(c)], op=MUL)
                nc.scalar.activation(compE[:, _c(c)], compL[:], EXP)

        def stage_b(h):
            OTu, recipZ, compE = OTu_t[h], recipZ_t[h], compE_t[h]
            # ---- CZ -> w8 = compE / (CZ * Z) ----
            w8 = hs.tile([8, T], BF16, tag="w8", name=f"w8{h}")
            for c in range(NC2):
                pcz = psS.tile([8, 512], F32, tag="small", name=f"pscz{h}_{c}")
                nc.tensor.matmul(pcz[:], ones88[:], compE[:, _c(c)], start=True, stop=True)
                recipCZ = scp.tile([8, 512], F32, tag="recipCZ", name=f"rcz{h}_{c}", bufs=1)
                nc.vector.reciprocal(recipCZ[:], pcz[:])
                denom = scp.tile([8, 512], F32, tag="denom", name=f"den{h}_{c}", bufs=1)
                nc.vector.tensor_tensor(denom[:], recipCZ[:], recipZ[:, _c(c)], op=MUL)
                nc.vector.tensor_tensor(w8[:, _c(c)], compE[:, _c(c)], denom[:], op=MUL)

            # ---- broadcast w (PE select-matmul) + weighted sum over rules ----
            for c in range(NC2):
                acc = None
                for r in range(R):
                    wr_ps = psA.tile([128, 512], F32, tag="acc", name=f"wrps{h}_{r}_{c}")
                    nc.tensor.matmul(wr_ps[:], sel8[:, _t(r)], w8[:, _c(c)], start=True, stop=True)
                    if r == 0:
                        acc = scp.tile([128, 512], BF16, tag=f"acc{c}a", name=f"ac{h}_{c}_0", bufs=1)
                        nc.vector.tensor_tensor(acc[:], wr_ps[:], OTu[:, r, _c(c)], op=MUL)
                    else:
                        prod = scp.tile([128, 512], BF16, tag=f"prod{c}", name=f"pr{h}_{c}_{r}")
                        nc.vector.tensor_tensor(prod[:], wr_ps[:], OTu[:, r, _c(c)], op=MUL)
                        if r < R - 1:
                            nacc = scp.tile([128, 512], BF16, tag=f"acc{c}{'b' if r % 2 else 'a'}", name=f"ac{h}_{c}_{r}", bufs=1)
                            nc.vector.tensor_tensor(nacc[:], acc[:], prod[:], op=ADD)
                            acc = nacc
                        else:
                            nc.vector.tensor_tensor(out2[:, h, _c(c)], acc[:], prod[:], op=ADD)

        # ---- software-pipelined head loop ----
        load_weights(0)
        for h in range(H):
            if h + 1 < H:
                load_weights(h + 1)
            stage_a(h)
            if h >= 1:
                stage_b(h - 1)
        stage_b(H - 1)

        # ---- merge: y = sum_h out2_h^T @ Wm_h ----
        for tt in range(TT):
            for c in range(NC2):
                py = psA.tile([128, 512], F32, tag="acc", name=f"psy{tt}_{c}")
                for h in range(H):
                    nc.tensor.matmul(
                        py[:], out2[:, h, _t(tt)], wm[:, h, _c(c)],
                        start=(h == 0), stop=(h == H - 1),
                    )
                yt = scp.tile([128, 512], F32, tag="yt", name=f"yt{tt}_{c}")
                nc.scalar.copy(yt[:], py[:])
                nc.sync.dma_start(io["y"][_t(tt), _c(c)], yt[:])


_CACHE = {}


def _build(flags_key):
    if flags_key in _CACHE:
        return _CACHE[flags_key]
    flags = dict(flags_key)
    nc = bacc.Bacc("TRN2", target_bir_lowering=False, debug=False, num_devices=NCORES)
    io = {}
    io["xT"] = nc.dram_tensor("xT", [D, T], BF16, kind="ExternalInput").ap()
    io["WqP"] = nc.dram_tensor("WqP", [H, 128, D], BF16, kind="ExternalInput").ap()
    io["WkP"] = nc.dram_tensor("WkP", [H, 128, D], BF16, kind="ExternalInput").ap()
    io["WqvP"] = nc.dram_tensor("WqvP", [H, 128, D], BF16, kind="ExternalInput").ap()
    io["WvP"] = nc.dram_tensor("WvP", [H, KT, 128, 1024], BF16, kind="ExternalInput").ap()
    io["WmP"] = nc.dram_tensor("WmP", [128, H * D], BF16, kind="ExternalInput").ap()
    io["c_wkvblk"] = nc.dram_tensor("c_wkvblk", [128, 512], BF16, kind="ExternalInput").ap()
    for bname, shape in (
        ("bq", [D]), ("bk", [D]), ("bv", [H * R * P]), ("bqv", [H * QK]), ("bkv", [QK]),
    ):
        if flags[bname]:
            dt = BF16 if bname == "bv" else F32
            io[bname] = nc.dram_tensor(bname, shape, dt, kind="ExternalInput").ap()
    io["c_ones8"] = nc.dram_tensor("c_ones8", [128, 8], BF16, kind="ExternalInput").ap()
    io["c_blkA"] = nc.dram_tensor("c_blkA", [128, 8], BF16, kind="ExternalInput").ap()
    io["c_blkB"] = nc.dram_tensor("c_blkB", [128, 8], BF16, kind="ExternalInput").ap()
    io["c_ones88"] = nc.dram_tensor("c_ones88", [8, 8], BF16, kind="ExternalInput").ap()
    io["c_sel8"] = nc.dram_tensor("c_sel8", [8, 1024], BF16, kind="ExternalInput").ap()
    if flags["bv"]:
        io["c_onesrow"] = nc.dram_tensor("c_onesrow", [1, 128], BF16, kind="ExternalInput").ap()
    io["y"] = nc.dram_tensor("y", [T, D], F32, kind="ExternalOutput").ap()

    with tile.TileContext(nc) as tc:
        build_kernel(tc, io, flags)
    nc.compile()
    _CACHE[flags_key] = (nc, flags)
    return _CACHE[flags_key]


def _wkvblk(Wkv):
    blk = np.zeros((128, 4, 128), np.float32)
    for g in range(4):
        blk[:, g, g * 32 : (g + 1) * 32] = Wkv
    return np.ascontiguousarray(blk.reshape(128, 512))


def _consts():
    ones8 = np.ones((128, 8), np.float32)
    blkA = np.zeros((128, 8), np.float32)
    blkB = np.zeros((128, 8), np.float32)
    for k in range(128):
        g = k // 32
        blkA[k, g] = 1.0
        blkB[k, 4 + g] = 1.0
    ones88 = np.ones((8, 8), np.float32)
    onesrow = np.ones((1, 128), np.float32)
    sel8 = np.zeros((8, 1024), np.float32)
    for m in range(1024):
        sel8[m // 128, m] = 1.0
    return {
        "c_ones8": ones8, "c_blkA": blkA, "c_blkB": blkB,
        "c_ones88": ones88, "c_onesrow": onesrow, "c_sel8": sel8,
    }


def _bf(a):
    return np.ascontiguousarray(a.astype(ml_dtypes.bfloat16))


def _pack_base(inp, flags):
    scale_q = np.float32(1.0 / np.sqrt(P))
    scale_qv = np.float32(1.0 / np.sqrt(QK))
    Wq_s = inp["Wq"] * scale_q
    Wqv_s = inp["Wqv"] * scale_qv
    # WqP[h,k,kt*128+m] = Wq_s[kt*128+k, h*128+m]
    WqP = Wq_s.reshape(KT, 128, H, 128).transpose(2, 1, 0, 3).reshape(H, 128, D)
    WkP = inp["Wk"].reshape(KT, 128, H, 128).transpose(2, 1, 0, 3).reshape(H, 128, D)
    # WqvP[h,k,kt*128+rep*32+j] = Wqv_s[kt*128+k, h*32+j]
    A = Wqv_s.reshape(KT, 128, H, QK).transpose(2, 1, 0, 3)  # [H,128,KT,QK]
    WqvP = np.broadcast_to(A[:, :, :, None, :], (H, 128, KT, 4, QK)).reshape(H, 128, D)
    # WvP[h,kt,k,rp] = Wv[kt*128+k, h*1024+rp]
    WvP = inp["Wv"].reshape(KT, 128, H, 1024).transpose(2, 0, 1, 3)
    # WmP[k, h*1024+d] = Wm[h*128+k, d]
    WmP = inp["Wm"].reshape(H, 128, D).transpose(1, 0, 2).reshape(128, H * D)
    consts = _consts()
    base = {
        "WqP": _bf(WqP), "WkP": _bf(WkP), "WqvP": _bf(WqvP),
        "WvP": _bf(WvP), "WmP": _bf(WmP),
        "c_wkvblk": _bf(_wkvblk(inp["Wkv"])),
        "c_ones8": _bf(consts["c_ones8"]), "c_blkA": _bf(consts["c_blkA"]),
        "c_blkB": _bf(consts["c_blkB"]), "c_ones88": _bf(consts["c_ones88"]),
        "c_sel8": _bf(consts["c_sel8"]),
    }
    if flags["bq"]:
        base["bq"] = np.ascontiguousarray(inp["bq"] * scale_q)
    if flags["bk"]:
        base["bk"] = np.ascontiguousarray(inp["bk"])
    if flags["bqv"]:
        base["bqv"] = np.ascontiguousarray(inp["bqv"] * scale_qv)
    if flags["bv"]:
        base["bv"] = _bf(inp["bv"])
        base["c_onesrow"] = _bf(consts["c_onesrow"])
    if flags["bkv"]:
        base["bkv"] = np.ascontiguousarray(inp["bkv"])
    return base


def _run(inputs, trace=False):
    inp = {k: np.ascontiguousarray(np.asarray(v, dtype=np.float32)) for k, v in inputs.items()}
    flags = {b: bool(np.any(inp[b])) for b in ("bq", "bk", "bv", "bqv", "bkv")}
    flags_key = tuple(sorted(flags.items()))
    nc, flags = _build(flags_key)
    base = _pack_base(inp, flags)
    in_maps = []
    for c in range(NCORES):
        m = dict(base)
        m["xT"] = _bf(inp["x"][c].T)
        in_maps.append(m)
    res = run_bass_kernel_spmd(nc, in_maps, list(range(NCORES)), trace=trace)
    out = np.stack([res.results[c]["y"] for c in range(NCORES)], axis=0)
    return out, res


def kernel(**inputs):
    out, _ = _run(inputs, trace=False)
    return out


def run_traced(inputs):
    """Like kernel() but with NTFF tracing; returns (out, BassKernelResults)."""
    return _run(inputs, trace=True)


# revision 4
# speedup vs baseline: 1.4886x; 1.1005x over previous
"""Trainium2 Bass kernel for CompositionalAttentionBase.

Problem (per batch element b, reference semantics):
  q = (x @ Wq + bq)  -> [T,H,P] * 1/sqrt(P)
  k = (x @ Wk + bk)  -> [T,H,P]
  v = (x @ Wv + bv)  -> [T,H,R,P]
  score = softmax(q k^T) per head            [H,Tq,Tk]
  out   = score @ v per (head, rule)         [T,H,R,P]
  q_v = (x @ Wqv + bqv)/sqrt(QK)             [T,H,QK]
  k_v = out @ Wkv + bkv                      [T,H,R,QK]
  comp = softmax_r(q_v . k_v)                [T,H,R]
  out2 = sum_r comp * out                    [T,H,P]
  y = out2.reshape(T,D) @ Wm

Sharding: pure data-parallel over batch. B == n_cores == 8, so each
NeuronCore computes one full batch element; no collectives at all.

v2 design notes (vs the fp32r baseline):
  - Everything on the PE is bf16 (PSUM accumulation stays fp32). At
    N=512 the matmul streams at 1 col/cycle for both fp32r and bf16,
    but bf16 enables fast-weight-load (64-cycle LDWEIGHTS, hidden
    behind the 512-cycle matmul) and halves all SBUF/DMA/evacuation
    traffic.
  - x is pre-transposed on the host (xT [D,T]) and all weights are
    pre-packed host-side into per-head contiguous layouts, so every
    weight load is one large DMA and the kernel does zero PE
    transposes.
  - The per-head program is split into stage A (projections, scores,
    V, retrieval OTu, composition logits) and stage B (composition
    softmax tail + rule-weighted sum). B(h-1) is emitted after A(h),
    so the PE never waits on the vector-engine softmax chain at a
    head boundary.
  - The rule-weighted sum uses a contiguous multiply/add chain on
    DVE instead of one big strided tensor_reduce (which measured
    ~15us per head in the baseline trace).

Per-core dataflow (head-by-head; all contractions natural TensorE
matmuls, scores kept unnormalized with 1/Z folded into the final
composition weights):
  qT_h = Wq_h^T @ xT        [P,T]     (Wq pre-scaled by 1/sqrt(P))
  kT_h = Wk_h^T @ xT        [P,T]
  ET   = exp(kT^T q-slices) [Tk,Tq]
  V_h  = xT^T @ Wv_h        [Tk,R*P]
  OTu_r = V_r^T @ ET        [P,Tq]    (unnormalized attention out)
  ZRep8 = ones8^T @ ET      [8,Tq]    -> recipZ
  qvT  = Wqv_h^T @ xT (4x row-replicated) [4*QK,T]
  kvT  = Wkv^T @ OTu_r (block-diag, 4 rules/psum) [4*QK,Tq]
  compU = blockdiag-sums of (kvT * qvT)   [8,Tq]
  compE = exp(compU / Z);  w = compE / (CZ * Z)
  out2_h = sum_r OTu_r * broadcast(w_r)   [P,T]
  y = sum_h out2_h^T @ Wm_h               [T,D]
"""

import numpy as np
import ml_dtypes

import concourse.bass as bass
import concourse.tile as tile
from concourse import bacc, mybir
from concourse.bass_utils import run_bass_kernel_spmd

B, T, D, H, R, QK = 8, 1024, 1024, 8, 8, 32
P = D // H  # 128
NCORES = 8
TT = T // 128  # 8 t-tiles
KT = D // 128  # 8 contraction tiles for D
NC2 = T // 512  # 2 free-dim chunks of 512 over T
F32 = mybir.dt.float32
BF16 = mybir.dt.bfloat16
EXP = mybir.ActivationFunctionType.Exp
MUL = mybir.AluOpType.mult
ADD = mybir.AluOpType.add


def _c(c):  # 512-chunk slice
    return slice(c * 512, (c + 1) * 512)


def _t(i):  # 128-tile slice
    return slice(i * 128, (i + 1) * 128)


def build_kernel(tc, io, flags):
    nc = tc.nc

    with (
        nc.allow_low_precision(reason="bf16 intermediates; end-to-end precision validated vs reference"),
        tc.tile_pool(name="cst", bufs=1) as cst,
        tc.tile_pool(name="per", bufs=1) as per,
        tc.tile_pool(name="hd", bufs=2) as hd,     # double-buffered per-head
        tc.tile_pool(name="hs", bufs=1) as hs,     # single-buffered per-head
        tc.tile_pool(name="sc", bufs=2) as scp,    # small vector scratch
        tc.tile_pool(name="psA", bufs=6, space="PSUM") as psA,
        tc.tile_pool(name="psS", bufs=2, space="PSUM") as psS,
    ):
        # ---- constants ----
        ones8 = cst.tile([128, 8], BF16, name="ones8")
        nc.sync.dma_start(ones8[:], io["c_ones8"])
        blkA = cst.tile([128, 8], BF16, name="blkA")
        nc.sync.dma_start(blkA[:], io["c_blkA"])
        blkB = cst.tile([128, 8], BF16, name="blkB")
        nc.sync.dma_start(blkB[:], io["c_blkB"])
        ones88 = cst.tile([8, 8], BF16, name="ones88")
        nc.sync.dma_start(ones88[:], io["c_ones88"])
        sel8 = cst.tile([8, 1024], BF16, name="sel8")
        nc.sync.dma_start(sel8[:], io["c_sel8"])
        wkvblk = cst.tile([128, 4, 128], BF16, name="wkvblk")
        nc.sync.dma_start(wkvblk[:], io["c_wkvblk"].rearrange("p (g m) -> p g m", g=4))
        if flags["bq"]:
            bq_sb = cst.tile([128, 8], F32, name="bq_sb")
            nc.sync.dma_start(bq_sb[:], io["bq"].rearrange("(h p) -> p h", p=128))
        if flags["bk"]:
            bk_sb = cst.tile([128, 8], F32, name="bk_sb")
            nc.sync.dma_start(bk_sb[:], io["bk"].rearrange("(h p) -> p h", p=128))
        if flags["bqv"]:
            bqv_sb = cst.tile([128, 8], F32, name="bqv_sb")
            nc.sync.dma_start(
                bqv_sb[:],
                io["bqv"].rearrange("(h q) -> q h", q=32).to_broadcast([4, 32, 8]).rearrange("r q h -> (r q) h"),
            )
        if flags["bv"]:
            onesrow = cst.tile([1, 128], BF16, name="onesrow")
            nc.sync.dma_start(onesrow[:], io["c_onesrow"])
        if flags["bkv"]:
            bkv_sb = cst.tile([128, 1], F32, name="bkv_sb")
            nc.sync.dma_start(
                bkv_sb[:], io["bkv"].rearrange("(o q) -> q o", o=1).to_broadcast([4, 32, 1]).rearrange("r q o -> (r q) o")
            )

        # ---- persistent tiles ----
        xT = per.tile([128, KT, T], BF16, name="xT")
        for kt in range(KT):
            nc.sync.dma_start(xT[:, kt], io["xT"][_t(kt), :])
        wm = per.tile([128, H, D], BF16, name="wm")
        nc.sync.dma_start(wm[:], io["WmP"].rearrange("k (h d) -> k h d", h=H))
        out2 = per.tile([128, H, T], BF16, name="out2")

        # ---- per-head weight loads (prefetched one head ahead) ----
        wq = [None] * H
        wk = [None] * H
        wqv = [None] * H
        wv = [None] * H

        def load_weights(h):
            wq[h] = hd.tile([128, D], BF16, tag="wq", name=f"wq{h}")
            nc.sync.dma_start(wq[h][:], io["WqP"][h])
            wk[h] = hd.tile([128, D], BF16, tag="wk", name=f"wk{h}")
            nc.sync.dma_start(wk[h][:], io["WkP"][h])
            wqv[h] = hd.tile([128, D], BF16, tag="wqv", name=f"wqv{h}")
            nc.sync.dma_start(wqv[h][:], io["WqvP"][h])
            wv[h] = hd.tile([128, KT, 1024], BF16, tag="wv", name=f"wv{h}")
            for kt in range(KT):
                nc.sync.dma_start(wv[h][:, kt], io["WvP"][h, kt])

        # per-head state handed from stage A to stage B
        OTu_t = [None] * H
        recipZ_t = [None] * H
        compE_t = [None] * H

        def stage_a(h, mid_cb=None):
            # ---- qT / kT ----
            qT = hd.tile([128, T], BF16, tag="qT", name=f"qT{h}")
            kT = hd.tile([128, T], BF16, tag="kT", name=f"kT{h}")
            for dst, w, bflag, bname in (
                (qT, wq[h], flags["bq"], "bq"),
                (kT, wk[h], flags["bk"], "bk"),
            ):
                for c in range(NC2):
                    ps = psA.tile([128, 512], F32, tag="acc", name=f"psqk{h}_{c}")
                    for kt in range(KT):
                        nc.tensor.matmul(
                            ps[:], w[:, _t(kt)], xT[:, kt, _c(c)],
                            start=(kt == 0), stop=(kt == KT - 1),
                        )
                    if bflag:
                        nc.scalar.activation(
                            dst[:, _c(c)], ps[:],
                            mybir.ActivationFunctionType.Identity,
                            bias=(bq_sb if dst is qT else bk_sb)[:, h : h + 1],
                        )
                    else:
                        nc.scalar.copy(dst[:, _c(c)], ps[:])

            # ---- ET = exp(scores^T) [128(tk), TT, T(q)] ----
            ET = hs.tile([128, TT, T], BF16, tag="ET", name=f"ET{h}")
            for tk in range(TT):
                for c in range(NC2):
                    ps = psA.tile([128, 512], F32, tag="acc", name=f"pse{h}_{tk}_{c}")
                    nc.tensor.matmul(ps[:], kT[:, _t(tk)], qT[:, _c(c)], start=True, stop=True)
                    nc.scalar.activation(ET[:, tk, _c(c)], ps[:], EXP)

            if mid_cb is not None:
                mid_cb()

            # ---- V [128(tk), TT, R*P] ----
            V = hs.tile([128, TT, 1024], BF16, tag="V", name=f"V{h}")
            for tt in range(TT):
                for c in range(2):
                    pv = psA.tile([128, 512], F32, tag="acc", name=f"psv{h}_{tt}_{c}")
                    for kt in range(KT):
                        nc.tensor.matmul(
                            pv[:], xT[:, kt, _t(tt)], wv[h][:, kt, _c(c)],
                            start=(kt == 0), stop=(kt == KT - 1 and not flags["bv"]),
                        )
                    if flags["bv"]:
                        bv_t = scp.tile([1, 512], BF16, tag="bv", name=f"bv{h}_{tt}_{c}")
                        nc.sync.dma_start(bv_t[:], io["bv"][None, h * 1024 + c * 512 : h * 1024 + (c + 1) * 512])
                        nc.tensor.matmul(pv[:], onesrow[:], bv_t[:], start=False, stop=True)
                    nc.vector.tensor_copy(V[:, tt, _c(c)], pv[:])

            # ---- OTu_r = V_r^T @ ET  [128(p), R, T(q)] ----
            OTu = hd.tile([128, R, T], BF16, tag="OTu", name=f"OTu{h}")
            OTu_t[h] = OTu
            for r in range(R):
                for c in range(NC2):
                    po = psA.tile([128, 512], F32, tag="acc", name=f"pso{h}_{r}_{c}")
                    for tk in range(TT):
                        nc.tensor.matmul(
                            po[:], V[:, tk, _t(r)], ET[:, tk, _c(c)],
                            start=(tk == 0), stop=(tk == TT - 1),
                        )
                    nc.scalar.copy(OTu[:, r, _c(c)], po[:])

            # ---- Z (softmax denominator) -> recipZ ----
            recipZ = hd.tile([8, T], F32, tag="recipZ", name=f"recipZ{h}")
            recipZ_t[h] = recipZ
            for c in range(NC2):
                pz = psS.tile([8, 512], F32, tag="small", name=f"psz{h}_{c}")
                for tk in range(TT):
                    nc.tensor.matmul(
                        pz[:], ones8[:], ET[:, tk, _c(c)],
                        start=(tk == 0), stop=(tk == TT - 1),
                    )
                nc.vector.reciprocal_approx_fast(recipZ[:, _c(c)], pz[:])

            # ---- qvRep [128(4x qk), T] ----
            qvRep = hs.tile([128, T], BF16, tag="qvRep", name=f"qvRep{h}")
            for c in range(NC2):
                pq = psA.tile([128, 512], F32, tag="acc", name=f"psq{h}_{c}")
                for kt in range(KT):
                    nc.tensor.matmul(
                        pq[:], wqv[h][:, _t(kt)], xT[:, kt, _c(c)],
                        start=(kt == 0), stop=(kt == KT - 1),
                    )
                if flags["bqv"]:
                    nc.scalar.activation(
                        qvRep[:, _c(c)], pq[:],
                        mybir.ActivationFunctionType.Identity,
                        bias=bqv_sb[:, h : h + 1],
                    )
                else:
                    nc.scalar.copy(qvRep[:, _c(c)], pq[:])

            # ---- kvT (4 rules / psum via block-diag Wkv) + P-mul ----
            PP = hs.tile([128, 2, T], BF16, tag="PP", name=f"PP{h}")
            for c in range(NC2):
                for g in range(2):
                    pk = psA.tile([128, 512], F32, tag="acc", name=f"psk{h}_{c}_{g}")
                    for rr in range(4):
                        r = g * 4 + rr
                        nc.tensor.matmul(
                            pk[:], wkvblk[:, rr], OTu[:, r, _c(c)],
                            start=(rr == 0), stop=(rr == 3),
                        )
                    if flags["bkv"]:
                        tmp = scp.tile([128, 512], F32, tag="kvtmp", name=f"kvt{h}_{c}_{g}")
                        nc.vector.tensor_scalar_add(tmp[:], pk[:], bkv_sb[:, 0:1])
                        nc.vector.tensor_tensor(PP[:, g, _c(c)], tmp[:], qvRep[:, _c(c)], op=MUL)
                    else:
                        nc.vector.tensor_tensor(PP[:, g, _c(c)], pk[:], qvRep[:, _c(c)], op=MUL)

            # ---- compU -> comp logits -> compE ----
            compE = hd.tile([8, T], BF16, tag="compE", name=f"compE{h}")
            compE_t[h] = compE
            for c in range(NC2):
                pc = psS.tile([8, 512], F32, tag="small", name=f"psc{h}_{c}")
                nc.tensor.matmul(pc[:], blkA[:], PP[:, 0, _c(c)], start=True, stop=False)
                nc.tensor.matmul(pc[:], blkB[:], PP[:, 1, _c(c)], start=False, stop=True)
                compL = scp.tile([8, 512], F32, tag="compL", name=f"compL{h}_{c}", bufs=1)
                nc.vector.tensor_tensor(compL[:], pc[:], recipZ[:, _c(c)], op=MUL)
                nc.scalar.activation(compE[:, _c(c)], compL[:], EXP)

        w8_t = [None] * H

        def stage_b1(h):
            recipZ, compE = recipZ_t[h], compE_t[h]
            # ---- CZ -> w8 = compE / (CZ * Z) ----
            w8 = hs.tile([8, T], BF16, tag="w8", name=f"w8{h}")
            w8_t[h] = w8
            for c in range(NC2):
                pcz = psS.tile([8, 512], F32, tag="small", name=f"pscz{h}_{c}")
                nc.tensor.matmul(pcz[:], ones88[:], compE[:, _c(c)], start=True, stop=True)
                recipCZ = scp.tile([8, 512], F32, tag="recipCZ", name=f"rcz{h}_{c}", bufs=1)
                nc.vector.reciprocal_approx_fast(recipCZ[:], pcz[:])
                denom = scp.tile([8, 512], F32, tag="denom", name=f"den{h}_{c}", bufs=1)
                nc.vector.tensor_tensor(denom[:], recipCZ[:], recipZ[:, _c(c)], op=MUL)
                nc.vector.tensor_tensor(w8[:, _c(c)], compE[:, _c(c)], denom[:], op=MUL)

        def stage_b2(h):
            OTu, w8 = OTu_t[h], w8_t[h]
            # ---- broadcast w (PE select-matmul) + weighted sum over rules ----
            for c in range(NC2):
                acc = None
                for r in range(R):
                    wr_ps = psA.tile([128, 512], F32, tag="acc", name=f"wrps{h}_{r}_{c}")
                    nc.tensor.matmul(wr_ps[:], sel8[:, _t(r)], w8[:, _c(c)], start=True, stop=True)
                    if r == 0:
                        acc = scp.tile([128, 512], BF16, tag=f"acc{c}a", name=f"ac{h}_{c}_0", bufs=1)
                        nc.vector.tensor_tensor(acc[:], wr_ps[:], OTu[:, r, _c(c)], op=MUL)
                    else:
                        prod = scp.tile([128, 512], BF16, tag=f"prod{c}", name=f"pr{h}_{c}_{r}")
                        nc.vector.tensor_tensor(prod[:], wr_ps[:], OTu[:, r, _c(c)], op=MUL)
                        if r < R - 1:
                            nacc = scp.tile([128, 512], BF16, tag=f"acc{c}{'b' if r % 2 else 'a'}", name=f"ac{h}_{c}_{r}", bufs=1)
                            nc.vector.tensor_tensor(nacc[:], acc[:], prod[:], op=ADD)
                            acc = nacc
                        else:
                            nc.vector.tensor_tensor(out2[:, h, _c(c)], acc[:], prod[:], op=ADD)

        # ---- software-pipelined head loop: B1(h-1) is emitted inside
        # A(h) after the ET stage (so its vector chain drains while the
        # PE runs A(h)'s big matmuls), B2(h-1) right after A(h).
        load_weights(0)
        for h in range(H):
            if h + 1 < H:
                load_weights(h + 1)
            stage_a(h, mid_cb=(lambda hh=h - 1: stage_b1(hh)) if h >= 1 else None)
            if h >= 1:
                stage_b2(h - 1)
        stage_b1(H - 1)
        stage_b2(H - 1)

        # ---- merge: y = sum_h out2_h^T @ Wm_h ----
        for tt in range(TT):
            for c in range(NC2):
                py = psA.tile([128, 512], F32, tag="acc", name=f"psy{tt}_{c}")
                for h in range(H):
                    nc.tensor.matmul(
                        py[:], out2[:, h, _t(tt)], wm[:, h, _c(c)],
                        start=(h == 0), stop=(h == H - 1),
                    )
                yt = scp.tile([128, 512], F32, tag="yt", name=f"yt{tt}_{c}")
                nc.scalar.copy(yt[:], py[:])
                nc.sync.dma_start(io["y"][_t(tt), _c(c)], yt[:])


_CACHE = {}


def _build(flags_key):
    if flags_key in _CACHE:
        return _CACHE[flags_key]
    flags = dict(flags_key)
    nc = bacc.Bacc("TRN2", target_bir_lowering=False, debug=False, num_devices=NCORES)
    io = {}
    io["xT"] = nc.dram_tensor("xT", [D, T], BF16, kind="ExternalInput").ap()
    io["WqP"] = nc.dram_tensor("WqP", [H, 128, D], BF16, kind="ExternalInput").ap()
    io["WkP"] = nc.dram_tensor("WkP", [H, 128, D], BF16, kind="ExternalInput").ap()
    io["WqvP"] = nc.dram_tensor("WqvP", [H, 128, D], BF16, kind="ExternalInput").ap()
    io["WvP"] = nc.dram_tensor("WvP", [H, KT, 128, 1024], BF16, kind="ExternalInput").ap()
    io["WmP"] = nc.dram_tensor("WmP", [128, H * D], BF16, kind="ExternalInput").ap()
    io["c_wkvblk"] = nc.dram_tensor("c_wkvblk", [128, 512], BF16, kind="ExternalInput").ap()
    for bname, shape in (
        ("bq", [D]), ("bk", [D]), ("bv", [H * R * P]), ("bqv", [H * QK]), ("bkv", [QK]),
    ):
        if flags[bname]:
            dt = BF16 if bname == "bv" else F32
            io[bname] = nc.dram_tensor(bname, shape, dt, kind="ExternalInput").ap()
    io["c_ones8"] = nc.dram_tensor("c_ones8", [128, 8], BF16, kind="ExternalInput").ap()
    io["c_blkA"] = nc.dram_tensor("c_blkA", [128, 8], BF16, kind="ExternalInput").ap()
    io["c_blkB"] = nc.dram_tensor("c_blkB", [128, 8], BF16, kind="ExternalInput").ap()
    io["c_ones88"] = nc.dram_tensor("c_ones88", [8, 8], BF16, kind="ExternalInput").ap()
    io["c_sel8"] = nc.dram_tensor("c_sel8", [8, 1024], BF16, kind="ExternalInput").ap()
    if flags["bv"]:
        io["c_onesrow"] = nc.dram_tensor("c_onesrow", [1, 128], BF16, kind="ExternalInput").ap()
    io["y"] = nc.dram_tensor("y", [T, D], F32, kind="ExternalOutput").ap()

    with tile.TileContext(nc) as tc:
        build_kernel(tc, io, flags)
    nc.compile()
    _CACHE[flags_key] = (nc, flags)
    return _CACHE[flags_key]


def _wkvblk(Wkv):
    blk = np.zeros((128, 4, 128), np.float32)
    for g in range(4):
        blk[:, g, g * 32 : (g + 1) * 32] = Wkv
    return np.ascontiguousarray(blk.reshape(128, 512))


def _consts():
    ones8 = np.ones((128, 8), np.float32)
    blkA = np.zeros((128, 8), np.float32)
    blkB = np.zeros((128, 8), np.float32)
    for k in range(128):
        g = k // 32
        blkA[k, g] = 1.0
        blkB[k, 4 + g] = 1.0
    ones88 = np.ones((8, 8), np.float32)
    onesrow = np.ones((1, 128), np.float32)
    sel8 = np.zeros((8, 1024), np.float32)
    for m in range(1024):
        sel8[m // 128, m] = 1.0
    return {
        "c_ones8": ones8, "c_blkA": blkA, "c_blkB": blkB,
        "c_ones88": ones88, "c_onesrow": onesrow, "c_sel8": sel8,
    }


def _bf(a):
    return np.ascontiguousarray(a.astype(ml_dtypes.bfloat16))


def _pack_base(inp, flags):
    scale_q = np.float32(1.0 / np.sqrt(P))
    scale_qv = np.float32(1.0 / np.sqrt(QK))
    Wq_s = inp["Wq"] * scale_q
    Wqv_s = inp["Wqv"] * scale_qv
    # WqP[h,k,kt*128+m] = Wq_s[kt*128+k, h*128+m]
    WqP = Wq_s.reshape(KT, 128, H, 128).transpose(2, 1, 0, 3).reshape(H, 128, D)
    WkP = inp["Wk"].reshape(KT, 128, H, 128).transpose(2, 1, 0, 3).reshape(H, 128, D)
    # WqvP[h,k,kt*128+rep*32+j] = Wqv_s[kt*128+k, h*32+j]
    A = Wqv_s.reshape(KT, 128, H, QK).transpose(2, 1, 0, 3)  # [H,128,KT,QK]
    WqvP = np.broadcast_to(A[:, :, :, None, :], (H, 128, KT, 4, QK)).reshape(H, 128, D)
    # WvP[h,kt,k,rp] = Wv[kt*128+k, h*1024+rp]
    WvP = inp["Wv"].reshape(KT, 128, H, 1024).transpose(2, 0, 1, 3)
    # WmP[k, h*1024+d] = Wm[h*128+k, d]
    WmP = inp["Wm"].reshape(H, 128, D).transpose(1, 0, 2).reshape(128, H * D)
    consts = _consts()
    base = {
        "WqP": _bf(WqP), "WkP": _bf(WkP), "WqvP": _bf(WqvP),
        "WvP": _bf(WvP), "WmP": _bf(WmP),
        "c_wkvblk": _bf(_wkvblk(inp["Wkv"])),
        "c_ones8": _bf(consts["c_ones8"]), "c_blkA": _bf(consts["c_blkA"]),
        "c_blkB": _bf(consts["c_blkB"]), "c_ones88": _bf(consts["c_ones88"]),
        "c_sel8": _bf(consts["c_sel8"]),
    }
    if flags["bq"]:
        base["bq"] = np.ascontiguousarray(inp["bq"] * scale_q)
    if flags["bk"]:
        base["bk"] = np.ascontiguousarray(inp["bk"])
    if flags["bqv"]:
        base["bqv"] = np.ascontiguousarray(inp["bqv"] * scale_qv)
    if flags["bv"]:
        base["bv"] = _bf(inp["bv"])
        base["c_onesrow"] = _bf(consts["c_onesrow"])
    if flags["bkv"]:
        base["bkv"] = np.ascontiguousarray(inp["bkv"])
    return base


def _run(inputs, trace=False):
    inp = {k: np.ascontiguousarray(np.asarray(v, dtype=np.float32)) for k, v in inputs.items()}
    flags = {b: bool(np.any(inp[b])) for b in ("bq", "bk", "bv", "bqv", "bkv")}
    flags_key = tuple(sorted(flags.items()))
    nc, flags = _build(flags_key)
    base = _pack_base(inp, flags)
    in_maps = []
    for c in range(NCORES):
        m = dict(base)
        m["xT"] = _bf(inp["x"][c].T)
        in_maps.append(m)
    res = run_bass_kernel_spmd(nc, in_maps, list(range(NCORES)), trace=trace)
    out = np.stack([res.results[c]["y"] for c in range(NCORES)], axis=0)
    return out, res


def kernel(**inputs):
    out, _ = _run(inputs, trace=False)
    return out


def run_traced(inputs):
    """Like kernel() but with NTFF tracing; returns (out, BassKernelResults)."""
    return _run(inputs, trace=True)


# revision 5
# speedup vs baseline: 1.5704x; 1.0549x over previous
"""Trainium2 Bass kernel for CompositionalAttentionBase.

Problem (per batch element b, reference semantics):
  q = (x @ Wq + bq)  -> [T,H,P] * 1/sqrt(P)
  k = (x @ Wk + bk)  -> [T,H,P]
  v = (x @ Wv + bv)  -> [T,H,R,P]
  score = softmax(q k^T) per head            [H,Tq,Tk]
  out   = score @ v per (head, rule)         [T,H,R,P]
  q_v = (x @ Wqv + bqv)/sqrt(QK)             [T,H,QK]
  k_v = out @ Wkv + bkv                      [T,H,R,QK]
  comp = softmax_r(q_v . k_v)                [T,H,R]
  out2 = sum_r comp * out                    [T,H,P]
  y = out2.reshape(T,D) @ Wm

Sharding: pure data-parallel over batch. B == n_cores == 8, so each
NeuronCore computes one full batch element; no collectives at all.

v2 design notes (vs the fp32r baseline):
  - Everything on the PE is bf16 (PSUM accumulation stays fp32). At
    N=512 the matmul streams at 1 col/cycle for both fp32r and bf16,
    but bf16 enables fast-weight-load (64-cycle LDWEIGHTS, hidden
    behind the 512-cycle matmul) and halves all SBUF/DMA/evacuation
    traffic.
  - x is pre-transposed on the host (xT [D,T]) and all weights are
    pre-packed host-side into per-head contiguous layouts, so every
    weight load is one large DMA and the kernel does zero PE
    transposes.
  - The per-head program is split into stage A (projections, scores,
    V, retrieval OTu, composition logits) and stage B (composition
    softmax tail + rule-weighted sum). B(h-1) is emitted after A(h),
    so the PE never waits on the vector-engine softmax chain at a
    head boundary.
  - The rule-weighted sum uses a contiguous multiply/add chain on
    DVE instead of one big strided tensor_reduce (which measured
    ~15us per head in the baseline trace).

Per-core dataflow (head-by-head; all contractions natural TensorE
matmuls, scores kept unnormalized with 1/Z folded into the final
composition weights):
  qT_h = Wq_h^T @ xT        [P,T]     (Wq pre-scaled by 1/sqrt(P))
  kT_h = Wk_h^T @ xT        [P,T]
  ET   = exp(kT^T q-slices) [Tk,Tq]
  V_h  = xT^T @ Wv_h        [Tk,R*P]
  OTu_r = V_r^T @ ET        [P,Tq]    (unnormalized attention out)
  ZRep8 = ones8^T @ ET      [8,Tq]    -> recipZ
  qvT  = Wqv_h^T @ xT (4x row-replicated) [4*QK,T]
  kvT  = Wkv^T @ OTu_r (block-diag, 4 rules/psum) [4*QK,Tq]
  compU = blockdiag-sums of (kvT * qvT)   [8,Tq]
  compE = exp(compU / Z);  w = compE / (CZ * Z)
  out2_h = sum_r OTu_r * broadcast(w_r)   [P,T]
  y = sum_h out2_h^T @ Wm_h               [T,D]
"""

import numpy as np
import ml_dtypes

import concourse.bass as bass
import concourse.tile as tile
from concourse import bacc, mybir
from concourse.bass_utils import run_bass_kernel_spmd

B, T, D, H, R, QK = 8, 1024, 1024, 8, 8, 32
P = D // H  # 128
NCORES = 8
TT = T // 128  # 8 t-tiles
KT = D // 128  # 8 contraction tiles for D
NC2 = T // 512  # 2 free-dim chunks of 512 over T
F32 = mybir.dt.float32
BF16 = mybir.dt.bfloat16
EXP = mybir.ActivationFunctionType.Exp
MUL = mybir.AluOpType.mult
ADD = mybir.AluOpType.add


def _c(c):  # 512-chunk slice
    return slice(c * 512, (c + 1) * 512)


def _t(i):  # 128-tile slice
    return slice(i * 128, (i + 1) * 128)


def build_kernel(tc, io, flags):
    nc = tc.nc

    with (
        nc.allow_low_precision(reason="bf16 intermediates; end-to-end precision validated vs reference"),
        tc.tile_pool(name="cst", bufs=1) as cst,
        tc.tile_pool(name="per", bufs=1) as per,
        tc.tile_pool(name="hd", bufs=2) as hd,     # double-buffered per-head
        tc.tile_pool(name="hs", bufs=1) as hs,     # single-buffered per-head
        tc.tile_pool(name="sc", bufs=2) as scp,    # small vector scratch
        tc.tile_pool(name="psA", bufs=6, space="PSUM") as psA,
        tc.tile_pool(name="psS", bufs=2, space="PSUM") as psS,
    ):
        # ---- constants ----
        ones8 = cst.tile([128, 8], BF16, name="ones8")
        nc.sync.dma_start(ones8[:], io["c_ones8"])
        blkA = cst.tile([128, 8], BF16, name="blkA")
        nc.sync.dma_start(blkA[:], io["c_blkA"])
        blkB = cst.tile([128, 8], BF16, name="blkB")
        nc.sync.dma_start(blkB[:], io["c_blkB"])
        ones88 = cst.tile([8, 8], BF16, name="ones88")
        nc.sync.dma_start(ones88[:], io["c_ones88"])
        sel8 = cst.tile([8, 1024], BF16, name="sel8")
        nc.sync.dma_start(sel8[:], io["c_sel8"])
        wkvblk = cst.tile([128, 4, 128], BF16, name="wkvblk")
        nc.sync.dma_start(wkvblk[:], io["c_wkvblk"].rearrange("p (g m) -> p g m", g=4))
        if flags["bq"]:
            bq_sb = cst.tile([128, 8], F32, name="bq_sb")
            nc.sync.dma_start(bq_sb[:], io["bq"].rearrange("(h p) -> p h", p=128))
        if flags["bk"]:
            bk_sb = cst.tile([128, 8], F32, name="bk_sb")
            nc.sync.dma_start(bk_sb[:], io["bk"].rearrange("(h p) -> p h", p=128))
        if flags["bqv"]:
            bqv_sb = cst.tile([128, 8], F32, name="bqv_sb")
            nc.sync.dma_start(
                bqv_sb[:],
                io["bqv"].rearrange("(h q) -> q h", q=32).to_broadcast([4, 32, 8]).rearrange("r q h -> (r q) h"),
            )
        if flags["bv"]:
            onesrow = cst.tile([1, 128], BF16, name="onesrow")
            nc.sync.dma_start(onesrow[:], io["c_onesrow"])
        if flags["bkv"]:
            bkv_sb = cst.tile([128, 1], F32, name="bkv_sb")
            nc.sync.dma_start(
                bkv_sb[:], io["bkv"].rearrange("(o q) -> q o", o=1).to_broadcast([4, 32, 1]).rearrange("r q o -> (r q) o")
            )

        # ---- persistent tiles ----
        xT = per.tile([128, KT, T], BF16, name="xT")
        for kt in range(KT):
            nc.sync.dma_start(xT[:, kt], io["xT"][_t(kt), :])
        wm = per.tile([128, H, D], BF16, name="wm")
        nc.sync.dma_start(wm[:], io["WmP"].rearrange("k (h d) -> k h d", h=H))
        out2 = per.tile([128, H, T], BF16, name="out2")

        # ---- per-head weight loads (prefetched one head ahead) ----
        wq = [None] * H
        wk = [None] * H
        wqv = [None] * H
        wv = [None] * H

        def load_weights(h):
            wq[h] = hd.tile([128, D], BF16, tag="wq", name=f"wq{h}")
            nc.sync.dma_start(wq[h][:], io["WqP"][h])
            wk[h] = hd.tile([128, D], BF16, tag="wk", name=f"wk{h}")
            nc.sync.dma_start(wk[h][:], io["WkP"][h])
            wqv[h] = hd.tile([128, D], BF16, tag="wqv", name=f"wqv{h}")
            nc.sync.dma_start(wqv[h][:], io["WqvP"][h])
            wv[h] = hd.tile([128, KT, 1024], BF16, tag="wv", name=f"wv{h}")
            for kt in range(KT):
                nc.sync.dma_start(wv[h][:, kt], io["WvP"][h, kt])

        # per-head state handed from stage A to stage B
        OTu_t = [None] * H
        recipZ_t = [None] * H
        compE_t = [None] * H

        def stage_a(h, after_et=None, mid_v=None, after_v=None):
            # ---- qT / kT ----
            qT = hd.tile([128, T], BF16, tag="qT", name=f"qT{h}")
            kT = hd.tile([128, T], BF16, tag="kT", name=f"kT{h}")
            for dst, w, bflag, bname in (
                (qT, wq[h], flags["bq"], "bq"),
                (kT, wk[h], flags["bk"], "bk"),
            ):
                for c in range(NC2):
                    ps = psA.tile([128, 512], F32, tag="acc", name=f"psqk{h}_{c}")
                    for kt in range(KT):
                        nc.tensor.matmul(
                            ps[:], w[:, _t(kt)], xT[:, kt, _c(c)],
                            start=(kt == 0), stop=(kt == KT - 1),
                        )
                    if bflag:
                        nc.scalar.activation(
                            dst[:, _c(c)], ps[:],
                            mybir.ActivationFunctionType.Identity,
                            bias=(bq_sb if dst is qT else bk_sb)[:, h : h + 1],
                        )
                    else:
                        nc.scalar.copy(dst[:, _c(c)], ps[:])

            # ---- ET = exp(scores^T) [128(tk), TT, T(q)] ----
            ET = hs.tile([128, TT, T], BF16, tag="ET", name=f"ET{h}")
            for tk in range(TT):
                for c in range(NC2):
                    ps = psA.tile([128, 512], F32, tag="acc", name=f"pse{h}_{tk}_{c}")
                    nc.tensor.matmul(ps[:], kT[:, _t(tk)], qT[:, _c(c)], start=True, stop=True)
                    nc.scalar.activation(ET[:, tk, _c(c)], ps[:], EXP)

            if after_et is not None:
                after_et()

            # ---- V [128(tk), TT, R*P] ----
            V = hs.tile([128, TT, 1024], BF16, tag="V", name=f"V{h}")
            for tt in range(TT):
                for c in range(2):
                    pv = psA.tile([128, 512], F32, tag="acc", name=f"psv{h}_{tt}_{c}")
                    for kt in range(KT):
                        nc.tensor.matmul(
                            pv[:], xT[:, kt, _t(tt)], wv[h][:, kt, _c(c)],
                            start=(kt == 0), stop=(kt == KT - 1 and not flags["bv"]),
                        )
                    if flags["bv"]:
                        bv_t = scp.tile([1, 512], BF16, tag="bv", name=f"bv{h}_{tt}_{c}")
                        nc.sync.dma_start(bv_t[:], io["bv"][None, h * 1024 + c * 512 : h * 1024 + (c + 1) * 512])
                        nc.tensor.matmul(pv[:], onesrow[:], bv_t[:], start=False, stop=True)
                    nc.vector.tensor_copy(V[:, tt, _c(c)], pv[:])
                if tt == 3 and mid_v is not None:
                    mid_v()
            if after_v is not None:
                after_v()

            # ---- OTu_r = V_r^T @ ET  [128(p), R, T(q)] ----
            OTu = hd.tile([128, R, T], BF16, tag="OTu", name=f"OTu{h}")
            OTu_t[h] = OTu
            for r in range(R):
                for c in range(NC2):
                    po = psA.tile([128, 512], F32, tag="acc", name=f"pso{h}_{r}_{c}")
                    for tk in range(TT):
                        nc.tensor.matmul(
                            po[:], V[:, tk, _t(r)], ET[:, tk, _c(c)],
                            start=(tk == 0), stop=(tk == TT - 1),
                        )
                    nc.scalar.copy(OTu[:, r, _c(c)], po[:])

            # ---- Z (softmax denominator) -> recipZ ----
            recipZ = hd.tile([8, T], F32, tag="recipZ", name=f"recipZ{h}")
            recipZ_t[h] = recipZ
            for c in range(NC2):
                pz = psS.tile([8, 512], F32, tag="small", name=f"psz{h}_{c}")
                for tk in range(TT):
                    nc.tensor.matmul(
                        pz[:], ones8[:], ET[:, tk, _c(c)],
                        start=(tk == 0), stop=(tk == TT - 1),
                    )
                nc.vector.reciprocal_approx_fast(recipZ[:, _c(c)], pz[:])

            # ---- qvRep [128(4x qk), T] ----
            qvRep = hs.tile([128, T], BF16, tag="qvRep", name=f"qvRep{h}")
            for c in range(NC2):
                pq = psA.tile([128, 512], F32, tag="acc", name=f"psq{h}_{c}")
                for kt in range(KT):
                    nc.tensor.matmul(
                        pq[:], wqv[h][:, _t(kt)], xT[:, kt, _c(c)],
                        start=(kt == 0), stop=(kt == KT - 1),
                    )
                if flags["bqv"]:
                    nc.scalar.activation(
                        qvRep[:, _c(c)], pq[:],
                        mybir.ActivationFunctionType.Identity,
                        bias=bqv_sb[:, h : h + 1],
                    )
                else:
                    nc.scalar.copy(qvRep[:, _c(c)], pq[:])

            # ---- kvT (4 rules / psum via block-diag Wkv) + P-mul ----
            PP = hs.tile([128, 2, T], BF16, tag="PP", name=f"PP{h}")
            for c in range(NC2):
                for g in range(2):
                    pk = psA.tile([128, 512], F32, tag="acc", name=f"psk{h}_{c}_{g}")
                    for rr in range(4):
                        r = g * 4 + rr
                        nc.tensor.matmul(
                            pk[:], wkvblk[:, rr], OTu[:, r, _c(c)],
                            start=(rr == 0), stop=(rr == 3),
                        )
                    if flags["bkv"]:
                        tmp = scp.tile([128, 512], F32, tag="kvtmp", name=f"kvt{h}_{c}_{g}")
                        nc.vector.tensor_scalar_add(tmp[:], pk[:], bkv_sb[:, 0:1])
                        nc.vector.tensor_tensor(PP[:, g, _c(c)], tmp[:], qvRep[:, _c(c)], op=MUL)
                    else:
                        nc.vector.tensor_tensor(PP[:, g, _c(c)], pk[:], qvRep[:, _c(c)], op=MUL)

            # ---- compU -> comp logits -> compE ----
            compE = hd.tile([8, T], BF16, tag="compE", name=f"compE{h}")
            compE_t[h] = compE
            for c in range(NC2):
                pc = psS.tile([8, 512], F32, tag="small", name=f"psc{h}_{c}")
                nc.tensor.matmul(pc[:], blkA[:], PP[:, 0, _c(c)], start=True, stop=False)
                nc.tensor.matmul(pc[:], blkB[:], PP[:, 1, _c(c)], start=False, stop=True)
                compL = scp.tile([8, 512], F32, tag="compL", name=f"compL{h}_{c}", bufs=1)
                nc.vector.tensor_tensor(compL[:], pc[:], recipZ[:, _c(c)], op=MUL)
                nc.scalar.activation(compE[:, _c(c)], compL[:], EXP)

        w8_t = [None] * H

        def stage_b1(h):
            recipZ, compE = recipZ_t[h], compE_t[h]
            # ---- CZ -> w8 = compE / (CZ * Z) ----
            w8 = hs.tile([8, T], BF16, tag="w8", name=f"w8{h}")
            w8_t[h] = w8
            for c in range(NC2):
                pcz = psS.tile([8, 512], F32, tag="small", name=f"pscz{h}_{c}")
                nc.tensor.matmul(pcz[:], ones88[:], compE[:, _c(c)], start=True, stop=True)
                recipCZ = scp.tile([8, 512], F32, tag="recipCZ", name=f"rcz{h}_{c}", bufs=1)
                nc.vector.reciprocal_approx_fast(recipCZ[:], pcz[:])
                denom = scp.tile([8, 512], F32, tag="denom", name=f"den{h}_{c}", bufs=1)
                nc.vector.tensor_tensor(denom[:], recipCZ[:], recipZ[:, _c(c)], op=MUL)
                nc.vector.tensor_tensor(w8[:, _c(c)], compE[:, _c(c)], denom[:], op=MUL)

        def stage_b2(h, c):
            OTu, w8 = OTu_t[h], w8_t[h]
            # ---- broadcast w (PE select-matmul) + weighted sum over rules ----
            if True:
                acc = None
                for r in range(R):
                    wr_ps = psA.tile([128, 512], F32, tag="acc", name=f"wrps{h}_{r}_{c}")
                    nc.tensor.matmul(wr_ps[:], sel8[:, _t(r)], w8[:, _c(c)], start=True, stop=True)
                    if r == 0:
                        acc = scp.tile([128, 512], BF16, tag=f"acc{c}a", name=f"ac{h}_{c}_0", bufs=1)
                        nc.vector.tensor_tensor(acc[:], wr_ps[:], OTu[:, r, _c(c)], op=MUL)
                    else:
                        prod = scp.tile([128, 512], BF16, tag=f"prod{c}", name=f"pr{h}_{c}_{r}")
                        nc.vector.tensor_tensor(prod[:], wr_ps[:], OTu[:, r, _c(c)], op=MUL)
                        if r < R - 1:
                            nacc = scp.tile([128, 512], BF16, tag=f"acc{c}{'b' if r % 2 else 'a'}", name=f"ac{h}_{c}_{r}", bufs=1)
                            nc.vector.tensor_tensor(nacc[:], acc[:], prod[:], op=ADD)
                            acc = nacc
                        else:
                            nc.vector.tensor_tensor(out2[:, h, _c(c)], acc[:], prod[:], op=ADD)

        def merge_tiles(tts):
            # y = sum_h out2_h^T @ Wm_h for the given t-tiles
            for tt in tts:
                for c in range(NC2):
                    py = psA.tile([128, 512], F32, tag="acc", name=f"psy{tt}_{c}")
                    for h in range(H):
                        nc.tensor.matmul(
                            py[:], out2[:, h, _t(tt)], wm[:, h, _c(c)],
                            start=(h == 0), stop=(h == H - 1),
                        )
                    yt = scp.tile([128, 512], F32, tag="yt", name=f"yt{tt}_{c}")
                    nc.scalar.copy(yt[:], py[:])
                    nc.sync.dma_start(io["y"][_t(tt), _c(c)], yt[:])

        # ---- software-pipelined head loop: the previous head's
        # composition tail (B1 = w8 chain, B2 = broadcast + weighted
        # sum) is emitted interleaved into this head's stage A so the
        # PE never throttles on the vector chain.
        load_weights(0)
        for h in range(H):
            if h + 1 < H:
                load_weights(h + 1)
            if h >= 1:
                stage_a(
                    h,
                    after_et=lambda hh=h - 1: stage_b1(hh),
                    mid_v=lambda hh=h - 1: stage_b2(hh, 0),
                    after_v=lambda hh=h - 1: stage_b2(hh, 1),
                )
            else:
                stage_a(h)
        # tail: last head's composition interleaved with the merge
        stage_b1(H - 1)
        stage_b2(H - 1, 0)
        merge_tiles(range(0, 4))
        stage_b2(H - 1, 1)
        merge_tiles(range(4, 8))


_CACHE = {}


def _build(flags_key):
    if flags_key in _CACHE:
        return _CACHE[flags_key]
    flags = dict(flags_key)
    nc = bacc.Bacc("TRN2", target_bir_lowering=False, debug=False, num_devices=NCORES)
    io = {}
    io["xT"] = nc.dram_tensor("xT", [D, T], BF16, kind="ExternalInput").ap()
    io["WqP"] = nc.dram_tensor("WqP", [H, 128, D], BF16, kind="ExternalInput").ap()
    io["WkP"] = nc.dram_tensor("WkP", [H, 128, D], BF16, kind="ExternalInput").ap()
    io["WqvP"] = nc.dram_tensor("WqvP", [H, 128, D], BF16, kind="ExternalInput").ap()
    io["WvP"] = nc.dram_tensor("WvP", [H, KT, 128, 1024], BF16, kind="ExternalInput").ap()
    io["WmP"] = nc.dram_tensor("WmP", [128, H * D], BF16, kind="ExternalInput").ap()
    io["c_wkvblk"] = nc.dram_tensor("c_wkvblk", [128, 512], BF16, kind="ExternalInput").ap()
    for bname, shape in (
        ("bq", [D]), ("bk", [D]), ("bv", [H * R * P]), ("bqv", [H * QK]), ("bkv", [QK]),
    ):
        if flags[bname]:
            dt = BF16 if bname == "bv" else F32
            io[bname] = nc.dram_tensor(bname, shape, dt, kind="ExternalInput").ap()
    io["c_ones8"] = nc.dram_tensor("c_ones8", [128, 8], BF16, kind="ExternalInput").ap()
    io["c_blkA"] = nc.dram_tensor("c_blkA", [128, 8], BF16, kind="ExternalInput").ap()
    io["c_blkB"] = nc.dram_tensor("c_blkB", [128, 8], BF16, kind="ExternalInput").ap()
    io["c_ones88"] = nc.dram_tensor("c_ones88", [8, 8], BF16, kind="ExternalInput").ap()
    io["c_sel8"] = nc.dram_tensor("c_sel8", [8, 1024], BF16, kind="ExternalInput").ap()
    if flags["bv"]:
        io["c_onesrow"] = nc.dram_tensor("c_onesrow", [1, 128], BF16, kind="ExternalInput").ap()
    io["y"] = nc.dram_tensor("y", [T, D], F32, kind="ExternalOutput").ap()

    with tile.TileContext(nc) as tc:
        build_kernel(tc, io, flags)
    nc.compile()
    _CACHE[flags_key] = (nc, flags)
    return _CACHE[flags_key]


def _wkvblk(Wkv):
    blk = np.zeros((128, 4, 128), np.float32)
    for g in range(4):
        blk[:, g, g * 32 : (g + 1) * 32] = Wkv
    return np.ascontiguousarray(blk.reshape(128, 512))


def _consts():
    ones8 = np.ones((128, 8), np.float32)
    blkA = np.zeros((128, 8), np.float32)
    blkB = np.zeros((128, 8), np.float32)
    for k in range(128):
        g = k // 32
        blkA[k, g] = 1.0
        blkB[k, 4 + g] = 1.0
    ones88 = np.ones((8, 8), np.float32)
    onesrow = np.ones((1, 128), np.float32)
    sel8 = np.zeros((8, 1024), np.float32)
    for m in range(1024):
        sel8[m // 128, m] = 1.0
    return {
        "c_ones8": ones8, "c_blkA": blkA, "c_blkB": blkB,
        "c_ones88": ones88, "c_onesrow": onesrow, "c_sel8": sel8,
    }


def _bf(a):
    return np.ascontiguousarray(a.astype(ml_dtypes.bfloat16))


def _pack_base(inp, flags):
    scale_q = np.float32(1.0 / np.sqrt(P))
    scale_qv = np.float32(1.0 / np.sqrt(QK))
    Wq_s = inp["Wq"] * scale_q
    Wqv_s = inp["Wqv"] * scale_qv
    # WqP[h,k,kt*128+m] = Wq_s[kt*128+k, h*128+m]
    WqP = Wq_s.reshape(KT, 128, H, 128).transpose(2, 1, 0, 3).reshape(H, 128, D)
    WkP = inp["Wk"].reshape(KT, 128, H, 128).transpose(2, 1, 0, 3).reshape(H, 128, D)
    # WqvP[h,k,kt*128+rep*32+j] = Wqv_s[kt*128+k, h*32+j]
    A = Wqv_s.reshape(KT, 128, H, QK).transpose(2, 1, 0, 3)  # [H,128,KT,QK]
    WqvP = np.broadcast_to(A[:, :, :, None, :], (H, 128, KT, 4, QK)).reshape(H, 128, D)
    # WvP[h,kt,k,rp] = Wv[kt*128+k, h*1024+rp]
    WvP = inp["Wv"].reshape(KT, 128, H, 1024).transpose(2, 0, 1, 3)
    # WmP[k, h*1024+d] = Wm[h*128+k, d]
    WmP = inp["Wm"].reshape(H, 128, D).transpose(1, 0, 2).reshape(128, H * D)
    consts = _consts()
    base = {
        "WqP": _bf(WqP), "WkP": _bf(WkP), "WqvP": _bf(WqvP),
        "WvP": _bf(WvP), "WmP": _bf(WmP),
        "c_wkvblk": _bf(_wkvblk(inp["Wkv"])),
        "c_ones8": _bf(consts["c_ones8"]), "c_blkA": _bf(consts["c_blkA"]),
        "c_blkB": _bf(consts["c_blkB"]), "c_ones88": _bf(consts["c_ones88"]),
        "c_sel8": _bf(consts["c_sel8"]),
    }
    if flags["bq"]:
        base["bq"] = np.ascontiguousarray(inp["bq"] * scale_q)
    if flags["bk"]:
        base["bk"] = np.ascontiguousarray(inp["bk"])
    if flags["bqv"]:
        base["bqv"] = np.ascontiguousarray(inp["bqv"] * scale_qv)
    if flags["bv"]:
        base["bv"] = _bf(inp["bv"])
        base["c_onesrow"] = _bf(consts["c_onesrow"])
    if flags["bkv"]:
        base["bkv"] = np.ascontiguousarray(inp["bkv"])
    return base


def _run(inputs, trace=False):
    inp = {k: np.ascontiguousarray(np.asarray(v, dtype=np.float32)) for k, v in inputs.items()}
    flags = {b: bool(np.any(inp[b])) for b in ("bq", "bk", "bv", "bqv", "bkv")}
    flags_key = tuple(sorted(flags.items()))
    nc, flags = _build(flags_key)
    base = _pack_base(inp, flags)
    in_maps = []
    for c in range(NCORES):
        m = dict(base)
        m["xT"] = _bf(inp["x"][c].T)
        in_maps.append(m)
    res = run_bass_kernel_spmd(nc, in_maps, list(range(NCORES)), trace=trace)
    out = np.stack([res.results[c]["y"] for c in range(NCORES)], axis=0)
    return out, res


def kernel(**inputs):
    out, _ = _run(inputs, trace=False)
    return out


def run_traced(inputs):
    """Like kernel() but with NTFF tracing; returns (out, BassKernelResults)."""
    return _run(inputs, trace=True)


# revision 7
# speedup vs baseline: 1.6007x; 1.0193x over previous
"""Trainium2 Bass kernel for CompositionalAttentionBase.

Problem (per batch element b, reference semantics):
  q = (x @ Wq + bq)  -> [T,H,P] * 1/sqrt(P)
  k = (x @ Wk + bk)  -> [T,H,P]
  v = (x @ Wv + bv)  -> [T,H,R,P]
  score = softmax(q k^T) per head            [H,Tq,Tk]
  out   = score @ v per (head, rule)         [T,H,R,P]
  q_v = (x @ Wqv + bqv)/sqrt(QK)             [T,H,QK]
  k_v = out @ Wkv + bkv                      [T,H,R,QK]
  comp = softmax_r(q_v . k_v)                [T,H,R]
  out2 = sum_r comp * out                    [T,H,P]
  y = out2.reshape(T,D) @ Wm

Sharding: pure data-parallel over batch. B == n_cores == 8, so each
NeuronCore computes one full batch element; no collectives at all.

v2 design notes (vs the fp32r baseline):
  - Everything on the PE is bf16 (PSUM accumulation stays fp32). At
    N=512 the matmul streams at 1 col/cycle for both fp32r and bf16,
    but bf16 enables fast-weight-load (64-cycle LDWEIGHTS, hidden
    behind the 512-cycle matmul) and halves all SBUF/DMA/evacuation
    traffic.
  - x is pre-transposed on the host (xT [D,T]) and all weights are
    pre-packed host-side into per-head contiguous layouts, so every
    weight load is one large DMA and the kernel does zero PE
    transposes.
  - The per-head program is split into stage A (projections, scores,
    V, retrieval OTu, composition logits) and stage B (composition
    softmax tail + rule-weighted sum). B(h-1) is emitted after A(h),
    so the PE never waits on the vector-engine softmax chain at a
    head boundary.
  - The rule-weighted sum uses a contiguous multiply/add chain on
    DVE instead of one big strided tensor_reduce (which measured
    ~15us per head in the baseline trace).

Per-core dataflow (head-by-head; all contractions natural TensorE
matmuls, scores kept unnormalized with 1/Z folded into the final
composition weights):
  qT_h = Wq_h^T @ xT        [P,T]     (Wq pre-scaled by 1/sqrt(P))
  kT_h = Wk_h^T @ xT        [P,T]
  ET   = exp(kT^T q-slices) [Tk,Tq]
  V_h  = xT^T @ Wv_h        [Tk,R*P]
  OTu_r = V_r^T @ ET        [P,Tq]    (unnormalized attention out)
  ZRep8 = ones8^T @ ET      [8,Tq]    -> recipZ
  qvT  = Wqv_h^T @ xT (4x row-replicated) [4*QK,T]
  kvT  = Wkv^T @ OTu_r (block-diag, 4 rules/psum) [4*QK,Tq]
  compU = blockdiag-sums of (kvT * qvT)   [8,Tq]
  compE = exp(compU / Z);  w = compE / (CZ * Z)
  out2_h = sum_r OTu_r * broadcast(w_r)   [P,T]
  y = sum_h out2_h^T @ Wm_h               [T,D]
"""

import numpy as np
import ml_dtypes

import concourse.bass as bass
import concourse.tile as tile
from concourse import bacc, mybir
from concourse.bass_utils import run_bass_kernel_spmd

B, T, D, H, R, QK = 8, 1024, 1024, 8, 8, 32
P = D // H  # 128
NCORES = 8
TT = T // 128  # 8 t-tiles
KT = D // 128  # 8 contraction tiles for D
NC2 = T // 512  # 2 free-dim chunks of 512 over T
F32 = mybir.dt.float32
BF16 = mybir.dt.bfloat16
EXP = mybir.ActivationFunctionType.Exp
MUL = mybir.AluOpType.mult
ADD = mybir.AluOpType.add


def _c(c):  # 512-chunk slice
    return slice(c * 512, (c + 1) * 512)


def _t(i):  # 128-tile slice
    return slice(i * 128, (i + 1) * 128)


def build_kernel(tc, io, flags):
    nc = tc.nc

    with (
        nc.allow_low_precision(reason="bf16 intermediates; end-to-end precision validated vs reference"),
        tc.tile_pool(name="cst", bufs=1) as cst,
        tc.tile_pool(name="per", bufs=1) as per,
        tc.tile_pool(name="hd", bufs=2) as hd,     # double-buffered per-head
        tc.tile_pool(name="hs", bufs=1) as hs,     # single-buffered per-head
        tc.tile_pool(name="sc", bufs=2) as scp,    # small vector scratch
        tc.tile_pool(name="psA", bufs=6, space="PSUM") as psA,
        tc.tile_pool(name="psS", bufs=2, space="PSUM") as psS,
    ):
        # ---- constants (DMAs emitted after the critical first-head
        # weight loads; none is read before the Z stage of head 0) ----
        ones8 = cst.tile([128, 8], BF16, name="ones8")
        blkA = cst.tile([128, 8], BF16, name="blkA")
        blkB = cst.tile([128, 8], BF16, name="blkB")
        ones88 = cst.tile([8, 8], BF16, name="ones88")
        sel8 = cst.tile([8, 1024], BF16, name="sel8")
        wkvblk = cst.tile([128, 4, 128], BF16, name="wkvblk")

        def load_consts():
            nc.sync.dma_start(ones8[:], io["c_ones8"])
            nc.sync.dma_start(blkA[:], io["c_blkA"])
            nc.sync.dma_start(blkB[:], io["c_blkB"])
            nc.sync.dma_start(ones88[:], io["c_ones88"])
            nc.sync.dma_start(sel8[:], io["c_sel8"])
            nc.sync.dma_start(wkvblk[:], io["c_wkvblk"].rearrange("p (g m) -> p g m", g=4))
        if flags["bq"]:
            bq_sb = cst.tile([128, 8], F32, name="bq_sb")
            nc.sync.dma_start(bq_sb[:], io["bq"].rearrange("(h p) -> p h", p=128))
        if flags["bk"]:
            bk_sb = cst.tile([128, 8], F32, name="bk_sb")
            nc.sync.dma_start(bk_sb[:], io["bk"].rearrange("(h p) -> p h", p=128))
        if flags["bqv"]:
            bqv_sb = cst.tile([128, 8], F32, name="bqv_sb")
            nc.sync.dma_start(
                bqv_sb[:],
                io["bqv"].rearrange("(h q) -> q h", q=32).to_broadcast([4, 32, 8]).rearrange("r q h -> (r q) h"),
            )
        if flags["bv"]:
            onesrow = cst.tile([1, 128], BF16, name="onesrow")
            nc.sync.dma_start(onesrow[:], io["c_onesrow"])
        if flags["bkv"]:
            bkv_sb = cst.tile([128, 1], F32, name="bkv_sb")
            nc.sync.dma_start(
                bkv_sb[:], io["bkv"].rearrange("(o q) -> q o", o=1).to_broadcast([4, 32, 1]).rearrange("r q o -> (r q) o")
            )

        # ---- persistent tiles (wm DMA deferred; see head loop) ----
        xT = per.tile([128, KT, T], BF16, name="xT")
        wm = per.tile([128, H, D], BF16, name="wm")
        out2 = per.tile([128, H, T], BF16, name="out2")

        # ---- per-head weight loads (prefetched one head ahead) ----
        wq = [None] * H
        wk = [None] * H
        wqv = [None] * H
        wv = [None] * H

        def load_weights(h):
            wq[h] = hd.tile([128, D], BF16, tag="wq", name=f"wq{h}")
            nc.sync.dma_start(wq[h][:], io["WqP"][h])
            wk[h] = hd.tile([128, D], BF16, tag="wk", name=f"wk{h}")
            nc.sync.dma_start(wk[h][:], io["WkP"][h])
            wqv[h] = hd.tile([128, D], BF16, tag="wqv", name=f"wqv{h}")
            nc.sync.dma_start(wqv[h][:], io["WqvP"][h])
            wv[h] = hd.tile([128, KT, 1024], BF16, tag="wv", name=f"wv{h}")
            for kt in range(KT):
                nc.sync.dma_start(wv[h][:, kt], io["WvP"][h, kt])

        # per-head state handed from stage A to stage B
        OTu_t = [None] * H
        recipZ_t = [None] * H
        compE_t = [None] * H

        def stage_a(h, after_et=None, mid_v=None, after_v=None):
            # ---- qT / kT ----
            qT = hd.tile([128, T], BF16, tag="qT", name=f"qT{h}")
            kT = hd.tile([128, T], BF16, tag="kT", name=f"kT{h}")
            for dst, w, bflag, bname in (
                (qT, wq[h], flags["bq"], "bq"),
                (kT, wk[h], flags["bk"], "bk"),
            ):
                for c in range(NC2):
                    ps = psA.tile([128, 512], F32, tag="acc", name=f"psqk{h}_{c}")
                    for kt in range(KT):
                        nc.tensor.matmul(
                            ps[:], w[:, _t(kt)], xT[:, kt, _c(c)],
                            start=(kt == 0), stop=(kt == KT - 1),
                        )
                    if bflag:
                        nc.scalar.activation(
                            dst[:, _c(c)], ps[:],
                            mybir.ActivationFunctionType.Identity,
                            bias=(bq_sb if dst is qT else bk_sb)[:, h : h + 1],
                        )
                    else:
                        nc.scalar.copy(dst[:, _c(c)], ps[:])

            # ---- ET = exp(scores^T) [128(tk), TT, T(q)] ----
            ET = hs.tile([128, TT, T], BF16, tag="ET", name=f"ET{h}")
            for tk in range(TT):
                for c in range(NC2):
                    ps = psA.tile([128, 512], F32, tag="acc", name=f"pse{h}_{tk}_{c}")
                    nc.tensor.matmul(ps[:], kT[:, _t(tk)], qT[:, _c(c)], start=True, stop=True)
                    nc.scalar.activation(ET[:, tk, _c(c)], ps[:], EXP)

            if after_et is not None:
                after_et()

            # ---- V [128(tk), TT, R*P] ----
            V = hs.tile([128, TT, 1024], BF16, tag="V", name=f"V{h}")
            for tt in range(TT):
                for c in range(2):
                    pv = psA.tile([128, 512], F32, tag="acc", name=f"psv{h}_{tt}_{c}")
                    for kt in range(KT):
                        nc.tensor.matmul(
                            pv[:], xT[:, kt, _t(tt)], wv[h][:, kt, _c(c)],
                            start=(kt == 0), stop=(kt == KT - 1 and not flags["bv"]),
                        )
                    if flags["bv"]:
                        bv_t = scp.tile([1, 512], BF16, tag="bv", name=f"bv{h}_{tt}_{c}")
                        nc.sync.dma_start(bv_t[:], io["bv"][None, h * 1024 + c * 512 : h * 1024 + (c + 1) * 512])
                        nc.tensor.matmul(pv[:], onesrow[:], bv_t[:], start=False, stop=True)
                    nc.vector.tensor_copy(V[:, tt, _c(c)], pv[:])
                if tt == 3 and mid_v is not None:
                    mid_v()
            if after_v is not None:
                after_v()

            # ---- OTu_r = V_r^T @ ET  [128(p), R, T(q)] ----
            OTu = hd.tile([128, R, T], BF16, tag="OTu", name=f"OTu{h}")
            OTu_t[h] = OTu
            for r in range(R):
                for c in range(NC2):
                    po = psA.tile([128, 512], F32, tag="acc", name=f"pso{h}_{r}_{c}")
                    for tk in range(TT):
                        nc.tensor.matmul(
                            po[:], V[:, tk, _t(r)], ET[:, tk, _c(c)],
                            start=(tk == 0), stop=(tk == TT - 1),
                        )
                    nc.scalar.copy(OTu[:, r, _c(c)], po[:])

            # ---- Z (softmax denominator) -> recipZ ----
            recipZ = hd.tile([8, T], F32, tag="recipZ", name=f"recipZ{h}")
            recipZ_t[h] = recipZ
            for c in range(NC2):
                pz = psS.tile([8, 512], F32, tag="small", name=f"psz{h}_{c}")
                for tk in range(TT):
                    nc.tensor.matmul(
                        pz[:], ones8[:], ET[:, tk, _c(c)],
                        start=(tk == 0), stop=(tk == TT - 1),
                    )
                nc.vector.reciprocal_approx_fast(recipZ[:, _c(c)], pz[:])

            # ---- qvRep [128(4x qk), T] ----
            qvRep = hs.tile([128, T], BF16, tag="qvRep", name=f"qvRep{h}")
            for c in range(NC2):
                pq = psA.tile([128, 512], F32, tag="acc", name=f"psq{h}_{c}")
                for kt in range(KT):
                    nc.tensor.matmul(
                        pq[:], wqv[h][:, _t(kt)], xT[:, kt, _c(c)],
                        start=(kt == 0), stop=(kt == KT - 1),
                    )
                if flags["bqv"]:
                    nc.scalar.activation(
                        qvRep[:, _c(c)], pq[:],
                        mybir.ActivationFunctionType.Identity,
                        bias=bqv_sb[:, h : h + 1],
                    )
                else:
                    nc.scalar.copy(qvRep[:, _c(c)], pq[:])

            # ---- kvT (4 rules / psum via block-diag Wkv) + P-mul ----
            PP = hs.tile([128, 2, T], BF16, tag="PP", name=f"PP{h}")
            for c in range(NC2):
                for g in range(2):
                    pk = psA.tile([128, 512], F32, tag="acc", name=f"psk{h}_{c}_{g}")
                    for rr in range(4):
                        r = g * 4 + rr
                        nc.tensor.matmul(
                            pk[:], wkvblk[:, rr], OTu[:, r, _c(c)],
                            start=(rr == 0), stop=(rr == 3),
                        )
                    if flags["bkv"]:
                        tmp = scp.tile([128, 512], F32, tag="kvtmp", name=f"kvt{h}_{c}_{g}")
                        nc.vector.tensor_scalar_add(tmp[:], pk[:], bkv_sb[:, 0:1])
                        nc.vector.tensor_tensor(PP[:, g, _c(c)], tmp[:], qvRep[:, _c(c)], op=MUL)
                    else:
                        nc.vector.tensor_tensor(PP[:, g, _c(c)], pk[:], qvRep[:, _c(c)], op=MUL)

            # ---- compU -> comp logits -> compE ----
            compE = hd.tile([8, T], BF16, tag="compE", name=f"compE{h}")
            compE_t[h] = compE
            for c in range(NC2):
                pc = psS.tile([8, 512], F32, tag="small", name=f"psc{h}_{c}")
                nc.tensor.matmul(pc[:], blkA[:], PP[:, 0, _c(c)], start=True, stop=False)
                nc.tensor.matmul(pc[:], blkB[:], PP[:, 1, _c(c)], start=False, stop=True)
                compL = scp.tile([8, 512], F32, tag="compL", name=f"compL{h}_{c}", bufs=1)
                nc.vector.tensor_tensor(compL[:], pc[:], recipZ[:, _c(c)], op=MUL)
                nc.scalar.activation(compE[:, _c(c)], compL[:], EXP)

        w8_t = [None] * H

        def stage_b1(h):
            recipZ, compE = recipZ_t[h], compE_t[h]
            # ---- CZ -> w8 = compE / (CZ * Z) ----
            w8 = hs.tile([8, T], BF16, tag="w8", name=f"w8{h}")
            w8_t[h] = w8
            for c in range(NC2):
                pcz = psS.tile([8, 512], F32, tag="small", name=f"pscz{h}_{c}")
                nc.tensor.matmul(pcz[:], ones88[:], compE[:, _c(c)], start=True, stop=True)
                recipCZ = scp.tile([8, 512], F32, tag="recipCZ", name=f"rcz{h}_{c}", bufs=1)
                nc.vector.reciprocal_approx_fast(recipCZ[:], pcz[:])
                denom = scp.tile([8, 512], F32, tag="denom", name=f"den{h}_{c}", bufs=1)
                nc.vector.tensor_tensor(denom[:], recipCZ[:], recipZ[:, _c(c)], op=MUL)
                nc.vector.tensor_tensor(w8[:, _c(c)], compE[:, _c(c)], denom[:], op=MUL)

        def stage_b2(h, c):
            OTu, w8 = OTu_t[h], w8_t[h]
            # ---- broadcast w (PE select-matmul) + weighted sum over rules ----
            if True:
                acc = None
                for r in range(R):
                    wr_ps = psA.tile([128, 512], F32, tag="acc", name=f"wrps{h}_{r}_{c}")
                    nc.tensor.matmul(wr_ps[:], sel8[:, _t(r)], w8[:, _c(c)], start=True, stop=True)
                    if r == 0:
                        acc = scp.tile([128, 512], BF16, tag=f"acc{c}a", name=f"ac{h}_{c}_0", bufs=1)
                        nc.vector.tensor_tensor(acc[:], wr_ps[:], OTu[:, r, _c(c)], op=MUL)
                    else:
                        prod = scp.tile([128, 512], BF16, tag=f"prod{c}", name=f"pr{h}_{c}_{r}")
                        nc.vector.tensor_tensor(prod[:], wr_ps[:], OTu[:, r, _c(c)], op=MUL)
                        if r < R - 1:
                            nacc = scp.tile([128, 512], BF16, tag=f"acc{c}{'b' if r % 2 else 'a'}", name=f"ac{h}_{c}_{r}", bufs=1)
                            nc.vector.tensor_tensor(nacc[:], acc[:], prod[:], op=ADD)
                            acc = nacc
                        else:
                            nc.vector.tensor_tensor(out2[:, h, _c(c)], acc[:], prod[:], op=ADD)

        def merge_tiles(tts):
            # y = sum_h out2_h^T @ Wm_h for the given t-tiles
            for tt in tts:
                for c in range(NC2):
                    py = psA.tile([128, 512], F32, tag="acc", name=f"psy{tt}_{c}")
                    for h in range(H):
                        nc.tensor.matmul(
                            py[:], out2[:, h, _t(tt)], wm[:, h, _c(c)],
                            start=(h == 0), stop=(h == H - 1),
                        )
                    yt = scp.tile([128, 512], F32, tag="yt", name=f"yt{tt}_{c}")
                    nc.scalar.copy(yt[:], py[:])
                    nc.sync.dma_start(io["y"][_t(tt), _c(c)], yt[:])

        # ---- software-pipelined head loop: the previous head's
        # composition tail (B1 = w8 chain, B2 = broadcast + weighted
        # sum) is emitted interleaved into this head's stage A so the
        # PE never throttles on the vector chain. The startup DMA order
        # is critical-path aware (Sync triggers serialize at ~0.6us
        # each): wq first, then xT, then the rest of head 0's weights;
        # constants after those; wm (merge weights) deferred to head 1;
        # later heads prefetch from inside the previous head's stage A.
        wq[0] = hd.tile([128, D], BF16, tag="wq", name="wq0")
        nc.sync.dma_start(wq[0][:], io["WqP"][0])
        for kt in range(KT):
            nc.sync.dma_start(xT[:, kt], io["xT"][_t(kt), :])
        wk[0] = hd.tile([128, D], BF16, tag="wk", name="wk0")
        nc.sync.dma_start(wk[0][:], io["WkP"][0])
        wqv[0] = hd.tile([128, D], BF16, tag="wqv", name="wqv0")
        nc.sync.dma_start(wqv[0][:], io["WqvP"][0])
        wv[0] = hd.tile([128, KT, 1024], BF16, tag="wv", name="wv0")
        for kt in range(KT):
            nc.sync.dma_start(wv[0][:, kt], io["WvP"][0, kt])
        load_consts()

        def prefetch(h):
            if h == 2:
                nc.sync.dma_start(wm[:], io["WmP"].rearrange("k (h d) -> k h d", h=H))
            if h < H:
                load_weights(h)

        for h in range(H):
            if h >= 1:
                stage_a(
                    h,
                    after_et=lambda hh=h - 1: stage_b1(hh),
                    mid_v=lambda hh=h: (stage_b2(hh - 1, 0), prefetch(hh + 1)),
                    after_v=lambda hh=h - 1: stage_b2(hh, 1),
                )
            else:
                stage_a(h, mid_v=lambda: prefetch(1))
        # tail: last head's composition interleaved with the merge
        stage_b1(H - 1)
        stage_b2(H - 1, 0)
        merge_tiles(range(0, 4))
        stage_b2(H - 1, 1)
        merge_tiles(range(4, 8))


_CACHE = {}


def _build(flags_key):
    if flags_key in _CACHE:
        return _CACHE[flags_key]
    flags = dict(flags_key)
    nc = bacc.Bacc("TRN2", target_bir_lowering=False, debug=False, num_devices=NCORES)
    io = {}
    io["xT"] = nc.dram_tensor("xT", [D, T], BF16, kind="ExternalInput").ap()
    io["WqP"] = nc.dram_tensor("WqP", [H, 128, D], BF16, kind="ExternalInput").ap()
    io["WkP"] = nc.dram_tensor("WkP", [H, 128, D], BF16, kind="ExternalInput").ap()
    io["WqvP"] = nc.dram_tensor("WqvP", [H, 128, D], BF16, kind="ExternalInput").ap()
    io["WvP"] = nc.dram_tensor("WvP", [H, KT, 128, 1024], BF16, kind="ExternalInput").ap()
    io["WmP"] = nc.dram_tensor("WmP", [128, H * D], BF16, kind="ExternalInput").ap()
    io["c_wkvblk"] = nc.dram_tensor("c_wkvblk", [128, 512], BF16, kind="ExternalInput").ap()
    for bname, shape in (
        ("bq", [D]), ("bk", [D]), ("bv", [H * R * P]), ("bqv", [H * QK]), ("bkv", [QK]),
    ):
        if flags[bname]:
            dt = BF16 if bname == "bv" else F32
            io[bname] = nc.dram_tensor(bname, shape, dt, kind="ExternalInput").ap()
    io["c_ones8"] = nc.dram_tensor("c_ones8", [128, 8], BF16, kind="ExternalInput").ap()
    io["c_blkA"] = nc.dram_tensor("c_blkA", [128, 8], BF16, kind="ExternalInput").ap()
    io["c_blkB"] = nc.dram_tensor("c_blkB", [128, 8], BF16, kind="ExternalInput").ap()
    io["c_ones88"] = nc.dram_tensor("c_ones88", [8, 8], BF16, kind="ExternalInput").ap()
    io["c_sel8"] = nc.dram_tensor("c_sel8", [8, 1024], BF16, kind="ExternalInput").ap()
    if flags["bv"]:
        io["c_onesrow"] = nc.dram_tensor("c_onesrow", [1, 128], BF16, kind="ExternalInput").ap()
    io["y"] = nc.dram_tensor("y", [T, D], F32, kind="ExternalOutput").ap()

    with tile.TileContext(nc) as tc:
        build_kernel(tc, io, flags)
    nc.compile()
    _CACHE[flags_key] = (nc, flags)
    return _CACHE[flags_key]


def _wkvblk(Wkv):
    blk = np.zeros((128, 4, 128), np.float32)
    for g in range(4):
        blk[:, g, g * 32 : (g + 1) * 32] = Wkv
    return np.ascontiguousarray(blk.reshape(128, 512))


def _consts():
    ones8 = np.ones((128, 8), np.float32)
    blkA = np.zeros((128, 8), np.float32)
    blkB = np.zeros((128, 8), np.float32)
    for k in range(128):
        g = k // 32
        blkA[k, g] = 1.0
        blkB[k, 4 + g] = 1.0
    ones88 = np.ones((8, 8), np.float32)
    onesrow = np.ones((1, 128), np.float32)
    sel8 = np.zeros((8, 1024), np.float32)
    for m in range(1024):
        sel8[m // 128, m] = 1.0
    return {
        "c_ones8": ones8, "c_blkA": blkA, "c_blkB": blkB,
        "c_ones88": ones88, "c_onesrow": onesrow, "c_sel8": sel8,
    }


def _bf(a):
    return np.ascontiguousarray(a.astype(ml_dtypes.bfloat16))


def _pack_base(inp, flags):
    scale_q = np.float32(1.0 / np.sqrt(P))
    scale_qv = np.float32(1.0 / np.sqrt(QK))
    Wq_s = inp["Wq"] * scale_q
    Wqv_s = inp["Wqv"] * scale_qv
    # WqP[h,k,kt*128+m] = Wq_s[kt*128+k, h*128+m]
    WqP = Wq_s.reshape(KT, 128, H, 128).transpose(2, 1, 0, 3).reshape(H, 128, D)
    WkP = inp["Wk"].reshape(KT, 128, H, 128).transpose(2, 1, 0, 3).reshape(H, 128, D)
    # WqvP[h,k,kt*128+rep*32+j] = Wqv_s[kt*128+k, h*32+j]
    A = Wqv_s.reshape(KT, 128, H, QK).transpose(2, 1, 0, 3)  # [H,128,KT,QK]
    WqvP = np.broadcast_to(A[:, :, :, None, :], (H, 128, KT, 4, QK)).reshape(H, 128, D)
    # WvP[h,kt,k,rp] = Wv[kt*128+k, h*1024+rp]
    WvP = inp["Wv"].reshape(KT, 128, H, 1024).transpose(2, 0, 1, 3)
    # WmP[k, h*1024+d] = Wm[h*128+k, d]
    WmP = inp["Wm"].reshape(H, 128, D).transpose(1, 0, 2).reshape(128, H * D)
    consts = _consts()
    base = {
        "WqP": _bf(WqP), "WkP": _bf(WkP), "WqvP": _bf(WqvP),
        "WvP": _bf(WvP), "WmP": _bf(WmP),
        "c_wkvblk": _bf(_wkvblk(inp["Wkv"])),
        "c_ones8": _bf(consts["c_ones8"]), "c_blkA": _bf(consts["c_blkA"]),
        "c_blkB": _bf(consts["c_blkB"]), "c_ones88": _bf(consts["c_ones88"]),
        "c_sel8": _bf(consts["c_sel8"]),
    }
    if flags["bq"]:
        base["bq"] = np.ascontiguousarray(inp["bq"] * scale_q)
    if flags["bk"]:
        base["bk"] = np.ascontiguousarray(inp["bk"])
    if flags["bqv"]:
        base["bqv"] = np.ascontiguousarray(inp["bqv"] * scale_qv)
    if flags["bv"]:
        base["bv"] = _bf(inp["bv"])
        base["c_onesrow"] = _bf(consts["c_onesrow"])
    if flags["bkv"]:
        base["bkv"] = np.ascontiguousarray(inp["bkv"])
    return base


def _run(inputs, trace=False):
    inp = {k: np.ascontiguousarray(np.asarray(v, dtype=np.float32)) for k, v in inputs.items()}
    flags = {b: bool(np.any(inp[b])) for b in ("bq", "bk", "bv", "bqv", "bkv")}
    flags_key = tuple(sorted(flags.items()))
    nc, flags = _build(flags_key)
    base = _pack_base(inp, flags)
    in_maps = []
    for c in range(NCORES):
        m = dict(base)
        m["xT"] = _bf(inp["x"][c].T)
        in_maps.append(m)
    res = run_bass_kernel_spmd(nc, in_maps, list(range(NCORES)), trace=trace)
    out = np.stack([res.results[c]["y"] for c in range(NCORES)], axis=0)
    return out, res


def kernel(**inputs):
    out, _ = _run(inputs, trace=False)
    return out


def run_traced(inputs):
    """Like kernel() but with NTFF tracing; returns (out, BassKernelResults)."""
    return _run(inputs, trace=True)
